# revision 1
# baseline (speedup 1.0000x reference)
"""Differential attention (B=2, T=2048, C=2048, 8 heads x 256) on 8 trn2 cores.

Sharding: tensor-parallel over the 8 effective heads — core h computes head h's
projections + attention and a partial output projection; host sums partials.

Per-core layouts (bf16 matmuls, f32 PSUM):
  xT      [C, B*T]      (host-transposed input, shared)
  wqkvT   [C, 768]      (head slice of wq|wk|wv, host-transposed)
  woT     [256, C]      (head slice of wo * (1-lambda_init), host-transposed)
  Q,K,V come out [tok, d]; Q/K rms-normed then transposed to [d, tok].
  Scores computed transposed: S.T[kk, q] = K_tile.T @ Q  -> exp -> P.T tiles
  feed PV matmul as lhsT directly (no P transpose). Ones-column on V gives the
  softmax denominator from the same matmul. Causal blocks skipped; diagonal
  blocks masked multiplicatively post-exp (scores bounded by +-sqrt(128), so
  softmax needs no max subtraction).
"""

import math
from contextlib import ExitStack

import numpy as np

# ---- problem constants (hardcoded per the harness contract) ----
B = 2
T = 2048
C = 2048
N_HEAD = 8
HEAD_DIM = 256
HALF = 128
LAMBDA_INIT = 0.8
RMS_EPS = 1.1920929e-07
N_CORES = 8

P = 128          # partitions
TOK_CHUNK = 512  # projection tok chunk (DMA granularity)

DEFAULT_OPTS = dict(
    att_chunk=256,       # attention q-chunk width (256 or 512)
    qk_tr="pe",          # "pe" | "dma": Q/K transpose path
    y_tr="pe",           # "pe" | "dma": y transpose path
    oproj_copy="alt",    # out-proj PSUM->SBUF evacuation: "act"|"dve"|"alt"
    psum=(3, 3, 2),      # banks: (proj, st, y) — must sum to <= 8
    tr_dma_engine="scalar",  # HWDGE queue for DMA transposes
    pt_bufs=5,           # P.T tile double-buffer depth
    xc_bufs=2,           # x chunk prefetch depth
    vcopy="dve",         # "act" | "dve": V PSUM->SBUF copy engine
    split_dma=False,     # split weight/first-chunk DMAs per c-tile (fast ramp)
    xc_first=True,       # issue the first x chunk's DMA before the weights
    osb_merge=True,      # one output-store DMA per tok block (vs per c-chunk)
    rms_batch=1,         # 1 | 2: tok-blocks sharing one Sqrt (fewer ACT table switches)
    rms_mode="newton",   # "sqrt" (ACT Sqrt) | "newton" (DVE-only rsqrt)
    narrow_top=True,     # compute only the valid half of the top causal row
    out_dma_alt=False,   # alternate output stores between sync/scalar queues
    tr_pool="st",        # "st" | "pp": PSUM pool used by PE transposes
    k_major=True,        # produce K in [d, tok] layout directly; rms-norm of K
                         # folded into the exp scale (per-partition AP)
    ramp_mini=False,     # dedicate a 128-tok mini DMA + q|v weight half to the
                         # very first block so PE starts ~2x earlier
    tail_split=False,    # stream the final block's stores per c-chunk
    rms_bufs=4,
    qn_bufs=6,
    y0_mult=2,
    ksq_eng="dve",       # "act" | "dve": engine computing k^2 (k_major)
    ktcopy_eng="dve",    # "act" | "dve": engine evacuating KT psum (k_major)
    ytr_pool="y",        # "st" | "y": PSUM pool for the y transposes
    ksq_src="sbuf",      # "psum" | "sbuf": k^2 input (sbuf frees KT psum sooner)
    chunk_order="asc",   # "asc" | "desc": attention q-chunk processing order
    wsplit=False,        # split wqkv DMA: q|v half first, k third deferred
)


def build_nc(c_dim, t_dim, b_dim, **opts):
    """Build the per-core Bass module. All shapes in tokens/channels."""
    import concourse.mybir as mybir
    import concourse.tile as tile
    from concourse import bacc
    from concourse.masks import make_identity, make_upper_triangular

    o = dict(DEFAULT_OPTS)
    o.update(opts)
    QCH = o["att_chunk"]
    jpc = QCH // P  # j-blocks per attention chunk

    dt = mybir.dt
    f32 = dt.float32
    bf16 = dt.bfloat16
    AF = mybir.ActivationFunctionType
    OP = mybir.AluOpType

    n_ctiles = c_dim // P            # contraction tiles over C
    ntok = b_dim * t_dim             # total token rows
    n_blocks_b = t_dim // P          # 128-tok blocks per batch
    n_qchunks = t_dim // QCH         # attention q chunks per batch
    blocks_per_chunk = TOK_CHUNK // P
    inv_sqrt_half = 1.0 / math.sqrt(HALF)
    VP = 272                         # V tile pitch (256 vals + 1 ones + pad)

    nc = bacc.Bacc()
    xt = nc.declare_dram_parameter("xt", [c_dim, ntok], bf16, isOutput=False)
    wqkv = nc.declare_dram_parameter("wqkv", [c_dim, 3 * HEAD_DIM], bf16, isOutput=False)
    wot = nc.declare_dram_parameter("wot", [HEAD_DIM, c_dim], bf16, isOutput=False)
    lamneg = nc.declare_dram_parameter("lamneg", [P, 1], f32, isOutput=False)
    out = nc.declare_dram_parameter("out", [ntok, c_dim], f32, isOutput=True)

    xt_r = xt.ap().rearrange("(i p) t -> p i t", p=P)      # [128, n_ctiles, ntok]
    wqkv_r = wqkv.ap().rearrange("(i p) n -> p i n", p=P)  # [128, n_ctiles, 768]
    wot_r = wot.ap().rearrange("(e p) n -> p e n", p=P)    # [128, 2, c_dim]

    tr_eng = nc.scalar if o["tr_dma_engine"] == "scalar" else nc.sync

    with tile.TileContext(nc) as tc:
        with ExitStack() as ctx:
            # ---- persistent SBUF ----
            const_pool = ctx.enter_context(tc.tile_pool(name="const", bufs=1))
            wqkv_sb = const_pool.tile([P, n_ctiles, 3 * HEAD_DIM], bf16, name="wqkv_sb")
            wot_sb = const_pool.tile([P, 2, c_dim], bf16, name="wot_sb")
            lam_sb = const_pool.tile([P, 1], f32, name="lam_sb")
            ident = const_pool.tile([P, P], bf16, name="ident")
            trimask = const_pool.tile([P, P], bf16, name="trimask")
            ones_sb = const_pool.tile([P, 1], bf16, name="ones_sb")
            nc.vector.memset(ones_sb[:], 1.0)

            # ---- pools ----
            xc_pool = ctx.enter_context(tc.tile_pool(name="xc", bufs=o["xc_bufs"]))

            xc0 = None
            xcmini = None
            if o["ramp_mini"] and o["k_major"]:
                # smallest possible ramp: 0.5 MB of x + the q|v weight half,
                # so block 0's matmuls can start and finish early
                xcmini = const_pool.tile([P, n_ctiles, P], bf16, name="xcmini")
                nc.sync.dma_start(xcmini[:], xt_r[:, :, 0:P])
                nc.sync.dma_start(wqkv_sb[:, :, 0:512], wqkv_r[:, :, 0:512])
                xc0 = xc_pool.tile([P, n_ctiles, TOK_CHUNK], bf16, tag="xc",
                                   name="xc0")
                nc.sync.dma_start(xc0[:], xt_r[:, :, 0:TOK_CHUNK])
                nc.sync.dma_start(wqkv_sb[:, :, 512:768], wqkv_r[:, :, 512:768])
                nc.sync.dma_start(wot_sb[:], wot_r[:])
            else:
                if o["xc_first"]:
                    # the very first DMA in the queue is the data the first
                    # matmul needs; weights follow immediately after
                    xc0 = xc_pool.tile([P, n_ctiles, TOK_CHUNK], bf16, tag="xc",
                                       name="xc0")
                    nc.sync.dma_start(xc0[:], xt_r[:, :, 0:TOK_CHUNK])
                if o["split_dma"]:
                    for i in range(n_ctiles):
                        nc.scalar.dma_start(wqkv_sb[:, i, :], wqkv_r[:, i, :])
                    nc.scalar.dma_start(wot_sb[:], wot_r[:])
                elif o["wsplit"]:
                    nc.sync.dma_start(wqkv_sb[:, :, 0:512], wqkv_r[:, :, 0:512])
                    nc.sync.dma_start(wqkv_sb[:, :, 512:768],
                                      wqkv_r[:, :, 512:768])
                    nc.sync.dma_start(wot_sb[:], wot_r[:])
                else:
                    nc.sync.dma_start(wqkv_sb[:], wqkv_r[:])
                    nc.sync.dma_start(wot_sb[:], wot_r[:])
            nc.sync.dma_start(lam_sb[:], lamneg.ap())
            make_identity(nc, ident[:])
            # 1.0 where kk <= q (partition <= free), else 0
            make_upper_triangular(nc, trimask[:], val=1.0, diag=True)
            qt_pool = ctx.enter_context(tc.tile_pool(name="qt", bufs=2))
            kt_pool = ctx.enter_context(tc.tile_pool(name="kt", bufs=2))
            ksq_pool = ctx.enter_context(tc.tile_pool(name="ksq", bufs=2))
            kscale_pool = ctx.enter_context(tc.tile_pool(name="kscale", bufs=2))
            v_pool = ctx.enter_context(tc.tile_pool(name="v", bufs=2))
            yt_pool = ctx.enter_context(tc.tile_pool(name="yt", bufs=2))
            pt_pool = ctx.enter_context(tc.tile_pool(name="pt", bufs=o["pt_bufs"]))
            y0_pool = ctx.enter_context(tc.tile_pool(name="y0", bufs=o["y0_mult"] * jpc))
            osb_pool = ctx.enter_context(tc.tile_pool(name="osb", bufs=3))
            qn_pool = ctx.enter_context(tc.tile_pool(name="qn", bufs=o["qn_bufs"]))
            sq_pool = ctx.enter_context(tc.tile_pool(name="sq", bufs=2))
            rms_pool = ctx.enter_context(tc.tile_pool(name="rms", bufs=o["rms_bufs"]))
            nproj, nst, ny = o["psum"]
            psum_proj = ctx.enter_context(
                tc.tile_pool(name="psum_proj", bufs=nproj, space="PSUM"))
            psum_st = ctx.enter_context(
                tc.tile_pool(name="psum_st", bufs=nst, space="PSUM"))
            psum_y = ctx.enter_context(
                tc.tile_pool(name="psum_y", bufs=ny, space="PSUM"))

            tr_psum = psum_st if o["tr_pool"] == "st" else psum_proj
            tr_tag = o["tr_pool"] if o["tr_pool"] == "st" else "pp"
            tr_shape = 256 if o["tr_pool"] == "st" else 512

            def pe_transpose(dst_ap, src_ap):
                trp = tr_psum.tile([P, tr_shape], bf16, tag=tr_tag,
                                   name="trp")[:, :P]
                nc.tensor.transpose(trp, src_ap, ident[:])
                nc.vector.tensor_copy(dst_ap, trp)

            def pe_transpose_y(dst_ap, src_ap):
                trp = psum_y.tile([P, 257], bf16, tag="y", name="trpy")[:, :P]
                nc.tensor.transpose(trp, src_ap, ident[:])
                nc.vector.tensor_copy(dst_ap, trp)

            def dma_transpose(dst_ap, src_ap):
                tr_eng.dma_start_transpose(out=dst_ap, in_=src_ap)

            tr_qk = pe_transpose if o["qk_tr"] == "pe" else dma_transpose
            tr_y = pe_transpose if o["y_tr"] == "pe" else dma_transpose
            if o["ytr_pool"] == "y":
                tr_y = pe_transpose_y

            for b in range(b_dim):
                qt_sb = qt_pool.tile([P, 2, t_dim], bf16, name=f"qt_b{b}", tag="qt")
                kt_sb = kt_pool.tile([P, 2, t_dim], bf16, name=f"kt_b{b}", tag="kt")
                v_sb = v_pool.tile([P, n_blocks_b, VP], bf16, name=f"v_b{b}", tag="v")
                yt_sb = yt_pool.tile([P, 2, t_dim], bf16, name=f"yt_b{b}", tag="yt")
                kscale_sb = (kscale_pool.tile([P, 2 * n_blocks_b], f32,
                                              name=f"ksc_b{b}", tag="ksc")
                             if o["k_major"] else None)

                # ================= projections =================
                RB = o["rms_batch"]
                for ch in range(t_dim // TOK_CHUNK):
                    tok0 = b * t_dim + ch * TOK_CHUNK
                    if b == 0 and ch == 0 and xc0 is not None:
                        xc = xc0
                    else:
                        xc = xc_pool.tile([P, n_ctiles, TOK_CHUNK], bf16, tag="xc")
                        if o["split_dma"] and b == 0 and ch == 0:
                            for i in range(n_ctiles):
                                nc.sync.dma_start(
                                    xc[:, i, :], xt_r[:, i, tok0:tok0 + TOK_CHUNK])
                        else:
                            nc.sync.dma_start(
                                xc[:], xt_r[:, :, tok0:tok0 + TOK_CHUNK])

                    if o["k_major"]:
                        # --- K projection straight into [d, tok] layout ---
                        # wqkv packs [q(256) | v(256) | k(256)]; lhsT slices of
                        # the k block give KT = wk_h @ x.T per d-tile (= view)
                        kssq = psum_proj.tile([P, 512], f32, tag="pp",
                                              name="kssq")[:, :8]
                        for v in range(2):
                            ktp = psum_proj.tile([P, 512], f32, tag="pp",
                                                 name="ktp")
                            for i in range(n_ctiles):
                                nc.tensor.matmul(
                                    ktp[:],
                                    wqkv_sb[:, i, 512 + v * P:512 + (v + 1) * P],
                                    xc[:, i, :],
                                    start=(i == 0), stop=(i == n_ctiles - 1))
                            ktdst = kt_sb[:, v, ch * TOK_CHUNK:(ch + 1) * TOK_CHUNK]
                            if o["ktcopy_eng"] == "act":
                                nc.scalar.copy(ktdst, ktp[:])
                            else:
                                nc.vector.tensor_copy(ktdst, ktp[:])
                            ksq = ksq_pool.tile([P, TOK_CHUNK], bf16, tag="ksq")
                            ksrc = ktdst if o["ksq_src"] == "sbuf" else ktp[:]
                            if o["ksq_eng"] == "act":
                                nc.scalar.activation(ksq[:], ksrc, AF.Square)
                            else:
                                nc.vector.tensor_tensor(ksq[:], ksrc, ksrc,
                                                        op=OP.mult)
                            for t in range(blocks_per_chunk):
                                nc.tensor.matmul(
                                    kssq[:, 2 * t + v:2 * t + v + 1],
                                    ksq[:, t * P:(t + 1) * P], ones_sb[:],
                                    start=True, stop=True)
                        # kscale = 1/sqrt(ssq + 128*eps)  (includes 1/sqrt(128))
                        ksl = kscale_sb[:, ch * 2 * blocks_per_chunk:
                                        (ch + 1) * 2 * blocks_per_chunk]
                        if o["rms_mode"] == "sqrt":
                            nc.vector.tensor_scalar(ksl, kssq[:], 1.0,
                                                    HALF * RMS_EPS, OP.mult, OP.add)
                            nc.scalar.activation(ksl, ksl, AF.Sqrt)
                            nc.vector.reciprocal(ksl, ksl)
                        else:
                            # rescale into the Newton seed's fit range, then
                            # multiply the 1/sqrt(128) back in at the end
                            km = rms_pool.tile([P, 8], f32, tag="rms", name="km")
                            nc.vector.tensor_scalar(km[:], kssq[:], 1.0 / HALF,
                                                    RMS_EPS, OP.mult, OP.add)
                            kt1 = rms_pool.tile([P, 8], f32, tag="rms", name="kt1")
                            nc.vector.tensor_tensor(kt1[:], km[:], km[:], op=OP.mult)
                            nc.vector.tensor_scalar(ksl, km[:], -1.47991565,
                                                    2.07556761, OP.mult, OP.add)
                            nc.vector.scalar_tensor_tensor(
                                ksl, kt1[:], 0.41306651, ksl, op0=OP.mult,
                                op1=OP.add)
                            nc.vector.tensor_scalar_max(ksl, ksl, 0.05)
                            for _ in range(2):
                                nc.vector.tensor_tensor(kt1[:], ksl, ksl,
                                                        op=OP.mult)
                                nc.vector.scalar_tensor_tensor(
                                    kt1[:], kt1[:], -0.5, km[:], op0=OP.mult,
                                    op1=OP.mult)
                                nc.vector.tensor_scalar(kt1[:], kt1[:], 1.0, 1.5,
                                                        OP.mult, OP.add)
                                nc.vector.tensor_tensor(ksl, ksl, kt1[:],
                                                        op=OP.mult)
                            nc.vector.tensor_scalar_mul(ksl, ksl, inv_sqrt_half)

                    NH = 2 if o["k_major"] else 4   # rms'd halves per block
                    for tl0 in range(0, blocks_per_chunk, RB):
                        group = []   # (tb, qkv0, qkv1)
                        rmsg = rms_pool.tile([P, NH * RB], f32, tag="rms")
                        for g in range(RB):
                            tl = tl0 + g
                            tb = ch * blocks_per_chunk + tl
                            if o["k_major"]:
                                # one bank: [q1 q2 | v]
                                qv = psum_proj.tile([P, 512], f32, tag="pp",
                                                    name="qv")
                                first_blk = (b == 0 and ch == 0 and tl == 0
                                             and xcmini is not None)
                                for i in range(n_ctiles):
                                    lhsT = (xcmini[:, i, :] if first_blk
                                            else xc[:, i, tl * P:(tl + 1) * P])
                                    nc.tensor.matmul(
                                        qv[:], lhsT,
                                        wqkv_sb[:, i, 0:512],
                                        start=(i == 0), stop=(i == n_ctiles - 1))
                                qkv0, qkv1 = qv, qv
                                halves = [qv[:, 0:128], qv[:, 128:256]]
                            else:
                                qkv0 = psum_proj.tile([P, 512], f32, tag="pp",
                                                      name="qkv0")[:, :384]
                                qkv1 = psum_proj.tile([P, 512], f32, tag="pp",
                                                      name="qkv1")[:, :384]
                                for i in range(n_ctiles):
                                    lhsT = xc[:, i, tl * P:(tl + 1) * P]
                                    nc.tensor.matmul(qkv0, lhsT, wqkv_sb[:, i, 0:384],
                                                     start=(i == 0), stop=(i == n_ctiles - 1))
                                    nc.tensor.matmul(qkv1, lhsT, wqkv_sb[:, i, 384:768],
                                                     start=(i == 0), stop=(i == n_ctiles - 1))
                                # layout: qkv0 = [q1 q2 k1], qkv1 = [k2 v]
                                halves = [qkv0[:, 0:128], qkv0[:, 128:256],
                                          qkv0[:, 256:384], qkv1[:, 0:128]]
                            for j, h in enumerate(halves):
                                sq = sq_pool.tile([P, P], bf16, tag="sq", name="sq")
                                nc.scalar.activation(
                                    sq[:], h, AF.Square,
                                    accum_out=rmsg[:, NH * g + j:NH * g + j + 1])
                            group.append((tb, qkv0, qkv1, halves))
                        # rms: ssq -> 1/sqrt(ssq/128 + eps), batched over the group
                        nc.vector.tensor_scalar(rmsg[:], rmsg[:], 1.0 / HALF,
                                                RMS_EPS, OP.mult, OP.add)
                        if o["rms_mode"] == "sqrt":
                            nc.scalar.activation(rmsg[:], rmsg[:], AF.Sqrt)
                            nc.vector.reciprocal(rmsg[:], rmsg[:])
                        else:
                            # DVE-only rsqrt: quadratic seed + 2 Newton steps.
                            # m concentrates near 0.8 for rms-normed randn
                            # inputs; seed is a least-squares quadratic fit of
                            # rsqrt on m in [0.3, 2.0], clamped for safety.
                            m = rmsg
                            yv = rms_pool.tile([P, NH * RB], f32, tag="rms")
                            t1 = rms_pool.tile([P, NH * RB], f32, tag="rms")
                            nc.vector.tensor_tensor(t1[:], m[:], m[:], op=OP.mult)
                            nc.vector.tensor_scalar(yv[:], m[:], -1.47991565, 2.07556761,
                                                    OP.mult, OP.add)
                            nc.vector.scalar_tensor_tensor(
                                yv[:], t1[:], 0.41306651, yv[:], op0=OP.mult,
                                op1=OP.add)
                            nc.vector.tensor_scalar_max(yv[:], yv[:], 0.05)
                            for _ in range(2):
                                nc.vector.tensor_tensor(t1[:], yv[:], yv[:],
                                                        op=OP.mult)
                                nc.vector.scalar_tensor_tensor(
                                    t1[:], t1[:], -0.5, m[:], op0=OP.mult,
                                    op1=OP.mult)
                                nc.vector.tensor_scalar(t1[:], t1[:], 1.0, 1.5,
                                                        OP.mult, OP.add)
                                nc.vector.tensor_tensor(yv[:], yv[:], t1[:],
                                                        op=OP.mult)
                            rmsg = yv
                        dests = [(qt_sb, 0), (qt_sb, 1), (kt_sb, 0), (kt_sb, 1)]
                        for g, (tb, qkv0, qkv1, halves) in enumerate(group):
                            for j, h in enumerate(halves):
                                qn = qn_pool.tile([P, P], bf16, tag="qn")
                                nc.vector.tensor_scalar_mul(
                                    qn[:], h, rmsg[:, NH * g + j:NH * g + j + 1])
                                dst, view = dests[j]
                                tr_qk(dst[:, view, tb * P:(tb + 1) * P], qn[:])
                            # V (+ ones column for the softmax denominator)
                            vsrc = (qkv1[:, 256:512] if o["k_major"]
                                    else qkv1[:, 128:384])
                            if o["vcopy"] == "act":
                                nc.scalar.copy(v_sb[:, tb, 0:256], vsrc)
                            else:
                                nc.vector.tensor_copy(v_sb[:, tb, 0:256], vsrc)
                            nc.vector.memset(v_sb[:, tb, 256:257], 1.0)

                # ================= attention =================
                cqi_order = (range(n_qchunks) if o["chunk_order"] == "asc"
                             else range(n_qchunks - 1, -1, -1))
                for cqi in cqi_order:
                    q0 = cqi * QCH
                    jmax = jpc * cqi + (jpc - 1)   # top kk-tile in this chunk
                    y0s = []
                    for v in range(2):
                        ys = [psum_y.tile([P, 257], f32, tag="y", name="ys")
                              for _ in range(jpc)]
                        for i in range(jmax + 1):
                            # jj0: first valid j-slot for this row (causal)
                            jj0 = max(0, i - jpc * cqi) if o["narrow_top"] else 0
                            w = QCH - jj0 * P
                            st = psum_st.tile([P, QCH], f32, tag="st",
                                              name="st")[:, :w]
                            nc.tensor.matmul(
                                st[:], kt_sb[:, v, i * P:(i + 1) * P],
                                qt_sb[:, v, q0 + jj0 * P:q0 + QCH],
                                start=True, stop=True)
                            pt = pt_pool.tile([P, QCH], bf16, tag="pt", name="pt")[:, :w]
                            if o["k_major"]:
                                nc.scalar.activation(
                                    pt[:], st[:], AF.Exp,
                                    scale=kscale_sb[:, 2 * i + v:2 * i + v + 1])
                            else:
                                nc.scalar.activation(pt[:], st[:], AF.Exp,
                                                     scale=inv_sqrt_half)
                            dj = i - jpc * cqi    # diagonal j-slot if >= 0
                            if dj >= 0:
                                nc.vector.tensor_tensor(
                                    pt[:, (dj - jj0) * P:(dj - jj0 + 1) * P],
                                    pt[:, (dj - jj0) * P:(dj - jj0 + 1) * P],
                                    trimask[:], op=OP.mult)
                            for jj in range(jj0, jpc):
                                j = jpc * cqi + jj
                                if i > j:
                                    continue
                                nc.tensor.matmul(
                                    ys[jj][:],
                                    pt[:, (jj - jj0) * P:(jj - jj0 + 1) * P],
                                    v_sb[:, i, 0:257],
                                    start=(i == 0), stop=(i == j))
                        # epilogue for this view
                        for jj in range(jpc):
                            j = jpc * cqi + jj
                            inv = rms_pool.tile([P, 1], f32, tag="inv")
                            nc.vector.reciprocal(inv[:], ys[jj][:, 256:257])
                            if v == 0:
                                y0 = y0_pool.tile([P, 256], f32, tag="y0")
                                nc.vector.tensor_scalar_mul(
                                    y0[:], ys[jj][:, 0:256], inv[:])
                                y0s.append(y0)
                            else:
                                sc2 = rms_pool.tile([P, 1], f32, tag="inv")
                                nc.vector.tensor_tensor(
                                    sc2[:], inv[:], lam_sb[:], op=OP.mult)
                                yf = qn_pool.tile([P, 256], bf16, tag="yf")
                                nc.vector.scalar_tensor_tensor(
                                    yf[:], ys[jj][:, 0:256], sc2[:], y0s[jj][:],
                                    op0=OP.mult, op1=OP.add)
                                for e in range(2):
                                    tr_y(yt_sb[:, e, j * P:(j + 1) * P],
                                         yf[:, e * P:(e + 1) * P])

                # ================= output projection (partial) =================
                for tb in range(n_blocks_b):
                    row0 = b * t_dim + tb * P
                    merged = o["osb_merge"] and not (
                        o["tail_split"] and b == b_dim - 1
                        and tb == n_blocks_b - 1)
                    orow = (osb_pool.tile([P, c_dim], f32, tag="orow",
                                          name="orow")
                            if merged else None)
                    for cc in range(c_dim // 512):
                        op_ps = psum_proj.tile([P, 512], f32, tag="pp", name="ops")
                        for e in range(2):
                            nc.tensor.matmul(
                                op_ps[:], yt_sb[:, e, tb * P:(tb + 1) * P],
                                wot_sb[:, e, cc * 512:(cc + 1) * 512],
                                start=(e == 0), stop=(e == 1))
                        osb = (orow[:, cc * 512:(cc + 1) * 512] if merged
                               else osb_pool.tile([P, 512], f32, tag="osb"))
                        oc = o["oproj_copy"]
                        if oc == "alt":
                            oc = "act" if (tb + cc) % 2 == 0 else "dve"
                        if oc == "act":
                            nc.scalar.copy(osb, op_ps[:])
                        else:
                            nc.vector.tensor_copy(osb, op_ps[:])
                        if not merged:
                            out_eng = nc.sync
                            if o["out_dma_alt"] and (tb + cc) % 2 == 1:
                                out_eng = nc.scalar
                            out_eng.dma_start(
                                out.ap()[row0:row0 + P,
                                         cc * 512:(cc + 1) * 512], osb)
                    if merged:
                        out_eng = nc.sync
                        if o["out_dma_alt"] and tb % 2 == 1:
                            out_eng = nc.scalar
                        out_eng.dma_start(
                            out.ap()[row0:row0 + P, :], orow[:])
    nc.compile()
    return nc


_NC_CACHE = {}
TRACE = False        # set True (e.g. from test.py) to capture an NTFF profile
LAST_RESULT = None   # BassKernelResults of the most recent run


def _get_nc(c_dim, t_dim, b_dim):
    key = (c_dim, t_dim, b_dim)
    if key not in _NC_CACHE:
        _NC_CACHE[key] = build_nc(c_dim, t_dim, b_dim)
    return _NC_CACHE[key]


def prep_inputs(x, wq, wk, wv, wo, lq1, lk1, lq2, lk2, k_major=None):
    """Host-side prep: per-core input maps."""
    import ml_dtypes

    if k_major is None:
        k_major = DEFAULT_OPTS["k_major"]

    bf16 = ml_dtypes.bfloat16
    b_dim, t_dim, c_dim = x.shape

    lam1 = np.exp(np.sum(lq1.astype(np.float64) * lk1.astype(np.float64)))
    lam2 = np.exp(np.sum(lq2.astype(np.float64) * lk2.astype(np.float64)))
    lam_full = np.float32(lam1 - lam2 + LAMBDA_INIT)

    xtb = np.ascontiguousarray(
        x.reshape(b_dim * t_dim, c_dim).T).astype(bf16)
    lamneg = np.full((P, 1), -lam_full, dtype=np.float32)

    in_maps = []
    for h in range(N_CORES):
        sl = slice(h * HEAD_DIM, (h + 1) * HEAD_DIM)
        parts = ([wq[sl].T, wv[sl].T, wk[sl].T] if k_major
                 else [wq[sl].T, wk[sl].T, wv[sl].T])
        wqkv_h = np.concatenate(parts, axis=1).astype(bf16)
        wqkv_h = np.ascontiguousarray(wqkv_h)
        wot_h = np.ascontiguousarray(
            (wo[:, sl] * (1.0 - LAMBDA_INIT)).T).astype(bf16)
        in_maps.append({
            "xt": xtb, "wqkv": wqkv_h, "wot": wot_h, "lamneg": lamneg,
        })
    return in_maps


_FN_CACHE = {}


def _get_callable(nc):
    """Build (once) a reusable jitted shard_map callable for this module —
    mirrors bass2jax.run_bass_via_pjrt's multi-core path, but cached so
    repeat kernel() calls skip retracing."""
    if id(nc) in _FN_CACHE:
        return _FN_CACHE[id(nc)]
    import jax
    from jax.sharding import Mesh, PartitionSpec, NamedSharding
    from jax.experimental.shard_map import shard_map
    import concourse.mybir as mybir
    import concourse.bass2jax as b2j

    b2j.install_neuronx_cc_hook()
    pname = nc.partition_id_tensor.name if nc.partition_id_tensor else None
    in_names, out_names, out_avals, zero_shapes = [], [], [], []
    for alloc in nc.m.functions[0].allocations:
        if not isinstance(alloc, mybir.MemoryLocationSet):
            continue
        name = alloc.memorylocations[0].name
        if alloc.kind == "ExternalInput":
            if name != pname:
                in_names.append(name)
        elif alloc.kind == "ExternalOutput":
            out_names.append(name)
            shape = tuple(alloc.tensor_shape)
            dtype = mybir.dt.np(alloc.dtype)
            out_avals.append(jax.core.ShapedArray(shape, dtype))
            zero_shapes.append((shape, dtype))
    n_params = len(in_names)
    all_in = in_names + out_names
    if pname is not None:
        all_in = all_in + [pname]

    def _body(*args):
        operands = list(args)
        if pname is not None:
            operands.append(b2j.partition_id_tensor())
        return tuple(b2j._bass_exec_p.bind(
            *operands,
            out_avals=tuple(out_avals),
            in_names=tuple(all_in),
            out_names=tuple(out_names),
            lowering_input_output_aliases=(),
            sim_require_finite=True,
            sim_require_nnan=True,
            nc=nc,
        ))

    devices = jax.devices()[:N_CORES]
    mesh = Mesh(np.asarray(devices), ("core",))
    nio = n_params + len(out_names)
    fn = jax.jit(shard_map(_body, mesh=mesh,
                           in_specs=(PartitionSpec("core"),) * nio,
                           out_specs=(PartitionSpec("core"),) * len(out_names),
                           check_rep=False),
                 donate_argnums=tuple(range(n_params, nio)), keep_unused=True)
    sh = NamedSharding(mesh, PartitionSpec("core"))
    entry = (fn, in_names, out_names, zero_shapes, sh)
    _FN_CACHE[id(nc)] = entry
    return entry


def kernel(x, wq, wk, wv, wo, lq1, lk1, lq2, lk2):
    b_dim, t_dim, c_dim = x.shape
    in_maps = prep_inputs(x, wq, wk, wv, wo, lq1, lk1, lq2, lk2)
    nc = _get_nc(c_dim, t_dim, b_dim)

    try:
        import jax
        fn, in_names, out_names, zero_shapes, sh = _get_callable(nc)
        concat_in = [
            np.concatenate([np.asarray(in_maps[c][n]) for c in range(N_CORES)],
                           axis=0) for n in in_names]
        concat_zeros = [np.zeros((N_CORES * s[0], *s[1:]), d)
                        for s, d in zero_shapes]
        dev_in = [jax.device_put(a, sh) for a in concat_in]
        dev_zero = [jax.device_put(a, sh) for a in concat_zeros]
        outs = fn(*dev_in, *dev_zero)
        arr = np.asarray(outs[out_names.index("out")])
        acc = arr.reshape(N_CORES, b_dim * t_dim, c_dim).sum(
            axis=0, dtype=np.float32)
    except Exception:
        from concourse.bass_utils import run_bass_kernel_spmd
        res = run_bass_kernel_spmd(nc, in_maps, list(range(N_CORES)),
                                   trace=TRACE)
        global LAST_RESULT
        LAST_RESULT = res
        acc = np.zeros((b_dim * t_dim, c_dim), dtype=np.float32)
        for h in range(N_CORES):
            acc += res.results[h]["out"]
    return acc.reshape(b_dim, t_dim, c_dim)



# revision 6
# speedup vs baseline: 1.0114x; 1.0114x over previous
"""Differential attention (B=2, T=2048, C=2048, 8 heads x 256) on 8 trn2 cores.

Sharding: tensor-parallel over the 8 effective heads — core h computes head h's
projections + attention and a partial output projection; host sums partials.

Projections run in fp8e4m3 with a 3-term residual (hi/lo planes of both x and
w, DoubleRow matmuls): (xh+xl)@(wh+wl) ~ xh@wh + (xh@wl + xl@wh), each pair of
128-contraction tiles fused into one DoubleRow instruction at 0.5 cycles/row —
25% fewer PE cycles than bf16 at bf16-level accuracy. Weights are host-scaled
by 64 into fp8's normal range; q/k absorb the scale in rms-norm (the Newton
rsqrt seed is refit for the 4096x mean-square), v stays 64x and the out-proj
weights carry 1/64.

Attention (scores S.T = K.T@Q -> exp -> P.T @ V with a ones-column giving the
softmax denominator) stays bf16: P = exp(s) spans e^-inf..e^11 which fp8
cannot represent, and bf16 q/k are needed for exp accuracy. Causal blocks
skipped; diagonal blocks masked multiplicatively post-exp.

Output is stored fp16 (halves store DMA); host sums the 8 partials in f32.
"""

import math
from contextlib import ExitStack

import numpy as np

# ---- problem constants (hardcoded per the harness contract) ----
B = 2
T = 2048
C = 2048
N_HEAD = 8
HEAD_DIM = 256
HALF = 128
LAMBDA_INIT = 0.8
RMS_EPS = 1.1920929e-07
N_CORES = 8

P = 128          # partitions
TOK_CHUNK = 512  # projection tok chunk (DMA granularity)
WSCALE = 64.0    # host-side weight scale into fp8 normal range
MSCALE = WSCALE * WSCALE  # mean-square scale (4096)

# Newton rsqrt seed: least-squares quadratic fit of rsqrt on m in
# MSCALE*[0.3, 2.0] (q/k mean-squares after the 64x weight scale), clamped.
RSQ_A = 2.07556761 / WSCALE
RSQ_B = -1.47991565 / (MSCALE * WSCALE)
RSQ_C = 0.41306651 / (MSCALE * MSCALE * WSCALE)
RSQ_CLAMP = 0.05 / WSCALE

DEFAULT_OPTS = dict(
    att_chunk=256,       # attention q-chunk width (256 or 512)
    oproj_copy="alt",    # out-proj PSUM->SBUF evacuation: "act"|"dve"|"alt"
    psum=(3, 3, 2),      # banks: (proj, st, y) — must sum to <= 8
    pt_bufs=5,           # P.T tile double-buffer depth
    xc_bufs=2,           # x chunk prefetch depth
    vcopy="dve",         # "act" | "dve": V PSUM->SBUF copy engine
    osb_merge=True,      # one output-store DMA per tok block (vs per c-chunk)
    narrow_top=True,     # compute only the valid half of the top causal row
    tr_pool="st",        # "st" | "pp": PSUM pool used by PE transposes
    rms_bufs=4,
    qn_bufs=6,
    y0_mult=2,
    ksq_eng="dve",       # "act" | "dve": engine computing k^2
    ktcopy_eng="dve",    # "act" | "dve": engine evacuating KT psum
    ytr_pool="y",        # "st" | "y": PSUM pool for the y transposes
    ksq_src="sbuf",      # "psum" | "sbuf": k^2 input
    chunk_order="asc",   # "asc" | "desc": attention q-chunk processing order
    out_eng="scalar",    # "scalar" | "sync": queue for output stores
    tail_split=False,    # stream the final block's stores per c-chunk
)


def build_nc(c_dim, t_dim, b_dim, **opts):
    """Build the per-core Bass module. All shapes in tokens/channels."""
    import concourse.mybir as mybir
    import concourse.tile as tile
    from concourse import bacc
    from concourse.masks import make_identity, make_upper_triangular

    o = dict(DEFAULT_OPTS)
    o.update(opts)
    QCH = o["att_chunk"]
    jpc = QCH // P  # j-blocks per attention chunk

    dt = mybir.dt
    f32 = dt.float32
    f16 = dt.float16
    bf16 = dt.bfloat16
    fp8 = dt.float8e4
    AF = mybir.ActivationFunctionType
    OP = mybir.AluOpType
    DR = mybir.MatmulPerfMode.DoubleRow

    n_ctiles = c_dim // P            # contraction tiles over C
    npairs = n_ctiles // 2
    ntok = b_dim * t_dim             # total token rows
    n_blocks_b = t_dim // P          # 128-tok blocks per batch
    n_qchunks = t_dim // QCH         # attention q chunks per batch
    blocks_per_chunk = TOK_CHUNK // P
    inv_sqrt_half = 1.0 / math.sqrt(HALF)
    VP = 272                         # V tile pitch (256 vals + 1 ones + pad)

    nc = bacc.Bacc()
    # x planes: [lo, hi]; w planes: [hi, lo] — cross-term DoubleRow pairs
    # (x_lo*w_hi + x_hi*w_lo) then use natural ascending slices on both.
    xt2 = nc.declare_dram_parameter("xt2", [2, c_dim, ntok], fp8, isOutput=False)
    # columns: [k(256) | q(256) | v(256)] — k first so the ramp can start
    wqkv2 = nc.declare_dram_parameter("wqkv2", [2, c_dim, 3 * HEAD_DIM], fp8,
                                      isOutput=False)
    wot = nc.declare_dram_parameter("wot", [HEAD_DIM, c_dim], bf16, isOutput=False)
    lamneg = nc.declare_dram_parameter("lamneg", [P, 1], f32, isOutput=False)
    out = nc.declare_dram_parameter("out", [ntok, c_dim], f16, isOutput=True)

    xt_r = xt2.ap().rearrange("v (i p) t -> p v i t", p=P)    # [128,2,nct,ntok]
    wq_r = wqkv2.ap().rearrange("v (i p) n -> p v i n", p=P)  # [128,2,nct,768]
    wot_r = wot.ap().rearrange("(e p) n -> p e n", p=P)       # [128, 2, c_dim]

    with tile.TileContext(nc) as tc:
        with ExitStack() as ctx:
            # ---- persistent SBUF ----
            const_pool = ctx.enter_context(tc.tile_pool(name="const", bufs=1))
            wqkv_sb = const_pool.tile([P, 2, n_ctiles, 3 * HEAD_DIM], fp8,
                                      name="wqkv_sb")
            wot_sb = const_pool.tile([P, 2, c_dim], bf16, name="wot_sb")
            lam_sb = const_pool.tile([P, 1], f32, name="lam_sb")
            ident = const_pool.tile([P, P], bf16, name="ident")
            trimask = const_pool.tile([P, P], bf16, name="trimask")
            ones_sb = const_pool.tile([P, 1], bf16, name="ones_sb")
            nc.vector.memset(ones_sb[:], 1.0)

            xc_pool = ctx.enter_context(tc.tile_pool(name="xc", bufs=o["xc_bufs"]))

            # ---- ramp: split small DMAs across queues so PE starts early ----
            # K-proj h2=0 / QV blocks 0-1 of chunk 0 read xcmini (256 tok);
            # the k weight columns ride the scalar queue in parallel.
            MINI = 256
            xcmini = const_pool.tile([P, 2, n_ctiles, MINI], fp8, name="xcmini")
            nc.sync.dma_start(xcmini[:], xt_r[:, :, :, 0:MINI])
            nc.scalar.dma_start(wqkv_sb[:, :, :, 0:256], wq_r[:, :, :, 0:256])
            nc.scalar.dma_start(wqkv_sb[:, 0, :, 256:768],
                                wq_r[:, 0, :, 256:768])
            xc0 = xc_pool.tile([P, 2, n_ctiles, TOK_CHUNK], fp8, tag="xc",
                               name="xc0")
            nc.sync.dma_start(xc0[:], xt_r[:, :, :, 0:TOK_CHUNK])
            nc.scalar.dma_start(wqkv_sb[:, 1, :, 256:768],
                                wq_r[:, 1, :, 256:768])
            nc.scalar.dma_start(wot_sb[:], wot_r[:])
            nc.scalar.dma_start(lam_sb[:], lamneg.ap())
            make_identity(nc, ident[:])
            # 1.0 where kk <= q (partition <= free), else 0
            make_upper_triangular(nc, trimask[:], val=1.0, diag=True)

            qt_pool = ctx.enter_context(tc.tile_pool(name="qt", bufs=2))
            kt_pool = ctx.enter_context(tc.tile_pool(name="kt", bufs=2))
            ksq_pool = ctx.enter_context(tc.tile_pool(name="ksq", bufs=2))
            kscale_pool = ctx.enter_context(tc.tile_pool(name="kscale", bufs=2))
            v_pool = ctx.enter_context(tc.tile_pool(name="v", bufs=2))
            yt_pool = ctx.enter_context(tc.tile_pool(name="yt", bufs=2))
            pt_pool = ctx.enter_context(tc.tile_pool(name="pt", bufs=o["pt_bufs"]))
            y0_pool = ctx.enter_context(tc.tile_pool(name="y0", bufs=o["y0_mult"] * jpc))
            osb_pool = ctx.enter_context(tc.tile_pool(name="osb", bufs=3))
            qn_pool = ctx.enter_context(tc.tile_pool(name="qn", bufs=o["qn_bufs"]))
            sq_pool = ctx.enter_context(tc.tile_pool(name="sq", bufs=2))
            rms_pool = ctx.enter_context(tc.tile_pool(name="rms", bufs=o["rms_bufs"]))
            nproj, nst, ny = o["psum"]
            psum_proj = ctx.enter_context(
                tc.tile_pool(name="psum_proj", bufs=nproj, space="PSUM"))
            psum_st = ctx.enter_context(
                tc.tile_pool(name="psum_st", bufs=nst, space="PSUM"))
            psum_y = ctx.enter_context(
                tc.tile_pool(name="psum_y", bufs=ny, space="PSUM"))

            tr_psum = psum_st if o["tr_pool"] == "st" else psum_proj
            tr_tag = o["tr_pool"] if o["tr_pool"] == "st" else "pp"
            tr_shape = QCH if o["tr_pool"] == "st" else 512

            def pe_transpose(dst_ap, src_ap):
                trp = tr_psum.tile([P, tr_shape], bf16, tag=tr_tag,
                                   name="trp")[:, :P]
                nc.tensor.transpose(trp, src_ap, ident[:])
                nc.vector.tensor_copy(dst_ap, trp)

            def pe_transpose_y(dst_ap, src_ap):
                trp = psum_y.tile([P, 257], bf16, tag="y", name="trpy")[:, :P]
                nc.tensor.transpose(trp, src_ap, ident[:])
                nc.vector.tensor_copy(dst_ap, trp)

            tr_qk = pe_transpose
            tr_y = pe_transpose_y if o["ytr_pool"] == "y" else pe_transpose

            def dr_proj(out_ap, x_ap, w_ap, x_stationary):
                """3-term residual fp8 accumulation into out_ap [128, N].

                x_ap(pl, i, sl): plane/ctile/slice accessor; same for w_ap.
                Planes: x [lo, hi], w [hi, lo]. Emits 1.5*npairs DoubleRow
                matmuls; caller's region gets start on the first, stop on the
                last.
                """
                calls = []
                for pr in range(npairs):  # hi @ hi, k-tile pairs
                    calls.append((x_ap(1, slice(2 * pr, 2 * pr + 2)),
                                  w_ap(0, slice(2 * pr, 2 * pr + 2))))
                for i in range(n_ctiles):  # x_lo@w_hi + x_hi@w_lo per tile
                    calls.append((x_ap(slice(0, 2), i),
                                  w_ap(slice(0, 2), i)))
                n = len(calls)
                for idx, (xs, ws) in enumerate(calls):
                    lhsT, rhs = (xs, ws) if x_stationary else (ws, xs)
                    nc.tensor.matmul(out_ap, lhsT, rhs,
                                     start=(idx == 0), stop=(idx == n - 1),
                                     perf_mode=DR)

            def rsqrt_newton(dst, m, tmp_pool, width):
                """DVE-only rsqrt on the MSCALE-shifted mean-square range.
                Returns the tile holding the result (may be a fresh tile)."""
                t1 = tmp_pool.tile([P, width], f32, tag="rms", name="rsq_t1")
                nc.vector.tensor_tensor(t1[:], m, m, op=OP.mult)
                nc.vector.tensor_scalar(dst, m, RSQ_B, RSQ_A, OP.mult, OP.add)
                nc.vector.scalar_tensor_tensor(dst, t1[:], RSQ_C, dst,
                                               op0=OP.mult, op1=OP.add)
                nc.vector.tensor_scalar_max(dst, dst, RSQ_CLAMP)
                for _ in range(2):
                    nc.vector.tensor_tensor(t1[:], dst, dst, op=OP.mult)
                    nc.vector.scalar_tensor_tensor(t1[:], t1[:], -0.5, m,
                                                   op0=OP.mult, op1=OP.mult)
                    nc.vector.tensor_scalar(t1[:], t1[:], 1.0, 1.5,
                                            OP.mult, OP.add)
                    nc.vector.tensor_tensor(dst, dst, t1[:], op=OP.mult)

            for b in range(b_dim):
                qt_sb = qt_pool.tile([P, 2, t_dim], bf16, name=f"qt_b{b}", tag="qt")
                kt_sb = kt_pool.tile([P, 2, t_dim], bf16, name=f"kt_b{b}", tag="kt")
                v_sb = v_pool.tile([P, n_blocks_b, VP], bf16, name=f"v_b{b}", tag="v")
                yt_sb = yt_pool.tile([P, 2, t_dim], bf16, name=f"yt_b{b}", tag="yt")
                kscale_sb = kscale_pool.tile([P, 2 * n_blocks_b], f32,
                                             name=f"ksc_b{b}", tag="ksc")

                # ================= projections =================
                for ch in range(t_dim // TOK_CHUNK):
                    tok0 = b * t_dim + ch * TOK_CHUNK
                    if b == 0 and ch == 0:
                        xc = xc0
                    else:
                        xc = xc_pool.tile([P, 2, n_ctiles, TOK_CHUNK], fp8,
                                          tag="xc")
                        nc.sync.dma_start(xc[:], xt_r[:, :, :, tok0:tok0 + TOK_CHUNK])

                    # --- K projection straight into [d, tok] layout ---
                    kssq = psum_proj.tile([P, 512], f32, tag="pp",
                                          name="kssq")[:, :8]
                    for v in range(2):
                        ktp = psum_proj.tile([P, 512], f32, tag="pp", name="ktp")
                        for h2 in range(TOK_CHUNK // 256):
                            ts = slice(h2 * 256, (h2 + 1) * 256)
                            ksrc_t = (xcmini if (b == 0 and ch == 0 and h2 == 0)
                                      else xc)
                            dr_proj(
                                ktp[:, ts],
                                lambda pl, i, ts=ts, ksrc_t=ksrc_t:
                                    ksrc_t[:, pl, i, ts],
                                lambda pl, i, v=v: wqkv_sb[:, pl, i,
                                                           v * P:(v + 1) * P],
                                x_stationary=False)
                        ktdst = kt_sb[:, v, ch * TOK_CHUNK:(ch + 1) * TOK_CHUNK]
                        if o["ktcopy_eng"] == "act":
                            nc.scalar.copy(ktdst, ktp[:])
                        else:
                            nc.vector.tensor_copy(ktdst, ktp[:])
                        ksq = ksq_pool.tile([P, TOK_CHUNK], bf16, tag="ksq")
                        ksrc = ktdst if o["ksq_src"] == "sbuf" else ktp[:]
                        if o["ksq_eng"] == "act":
                            nc.scalar.activation(ksq[:], ksrc, AF.Square)
                        else:
                            nc.vector.tensor_tensor(ksq[:], ksrc, ksrc,
                                                    op=OP.mult)
                        for t in range(blocks_per_chunk):
                            nc.tensor.matmul(
                                kssq[:, 2 * t + v:2 * t + v + 1],
                                ksq[:, t * P:(t + 1) * P], ones_sb[:],
                                start=True, stop=True)
                    # kscale = (1/64)/sqrt(mean(k^2) + eps) * inv_sqrt_half
                    ksl = kscale_sb[:, ch * 2 * blocks_per_chunk:
                                    (ch + 1) * 2 * blocks_per_chunk]
                    km = rms_pool.tile([P, 8], f32, tag="rms", name="km")
                    nc.vector.tensor_scalar(km[:], kssq[:], 1.0 / HALF,
                                            MSCALE * RMS_EPS, OP.mult, OP.add)
                    rsqrt_newton(ksl, km[:], rms_pool, 8)
                    nc.vector.tensor_scalar_mul(ksl, ksl, inv_sqrt_half)

                    for tl in range(blocks_per_chunk):
                        tb = ch * blocks_per_chunk + tl
                        # one bank: [q1 q2 | v]
                        qv = psum_proj.tile([P, 512], f32, tag="pp", name="qv")
                        first_blk = (b == 0 and ch == 0 and tl < 2)
                        xsrc = xcmini if first_blk else xc
                        tsl = slice(tl * P, (tl + 1) * P)
                        for h2 in range(2):
                            dr_proj(
                                qv[:, h2 * 256:(h2 + 1) * 256],
                                lambda pl, i, xsrc=xsrc, tsl=tsl:
                                    xsrc[:, pl, i, tsl],
                                lambda pl, i, h2=h2: wqkv_sb[
                                    :, pl, i,
                                    256 + h2 * 256:256 + (h2 + 1) * 256],
                                x_stationary=True)
                        halves = [qv[:, 0:128], qv[:, 128:256]]
                        rmsg = rms_pool.tile([P, 2], f32, tag="rms")
                        for j, h in enumerate(halves):
                            sq = sq_pool.tile([P, P], bf16, tag="sq", name="sq")
                            nc.scalar.activation(
                                sq[:], h, AF.Square,
                                accum_out=rmsg[:, j:j + 1])
                        nc.vector.tensor_scalar(rmsg[:], rmsg[:], 1.0 / HALF,
                                                MSCALE * RMS_EPS, OP.mult, OP.add)
                        yv = rms_pool.tile([P, 2], f32, tag="rms")
                        rsqrt_newton(yv[:], rmsg[:], rms_pool, 2)
                        for j, h in enumerate(halves):
                            qn = qn_pool.tile([P, P], bf16, tag="qn")
                            nc.vector.tensor_scalar_mul(qn[:], h, yv[:, j:j + 1])
                            tr_qk(qt_sb[:, j, tb * P:(tb + 1) * P], qn[:])
                        # V (+ ones column for the softmax denominator)
                        vsrc = qv[:, 256:512]
                        if o["vcopy"] == "act":
                            nc.scalar.copy(v_sb[:, tb, 0:256], vsrc)
                        else:
                            nc.vector.tensor_copy(v_sb[:, tb, 0:256], vsrc)
                        nc.vector.memset(v_sb[:, tb, 256:257], 1.0)

                # ================= attention =================
                cqi_order = (range(n_qchunks) if o["chunk_order"] == "asc"
                             else range(n_qchunks - 1, -1, -1))
                for cqi in cqi_order:
                    q0 = cqi * QCH
                    jmax = jpc * cqi + (jpc - 1)   # top kk-tile in this chunk
                    y0s = []
                    for v in range(2):
                        ys = [psum_y.tile([P, 257], f32, tag="y", name="ys")
                              for _ in range(jpc)]
                        for i in range(jmax + 1):
                            # jj0: first valid j-slot for this row (causal)
                            jj0 = max(0, i - jpc * cqi) if o["narrow_top"] else 0
                            w = QCH - jj0 * P
                            st = psum_st.tile([P, QCH], f32, tag="st",
                                              name="st")[:, :w]
                            nc.tensor.matmul(
                                st[:], kt_sb[:, v, i * P:(i + 1) * P],
                                qt_sb[:, v, q0 + jj0 * P:q0 + QCH],
                                start=True, stop=True)
                            pt = pt_pool.tile([P, QCH], bf16, tag="pt", name="pt")[:, :w]
                            nc.scalar.activation(
                                pt[:], st[:], AF.Exp,
                                scale=kscale_sb[:, 2 * i + v:2 * i + v + 1])
                            dj = i - jpc * cqi    # diagonal j-slot if >= 0
                            if dj >= 0:
                                nc.vector.tensor_tensor(
                                    pt[:, (dj - jj0) * P:(dj - jj0 + 1) * P],
                                    pt[:, (dj - jj0) * P:(dj - jj0 + 1) * P],
                                    trimask[:], op=OP.mult)
                            for jj in range(jj0, jpc):
                                j = jpc * cqi + jj
                                if i > j:
                                    continue
                                nc.tensor.matmul(
                                    ys[jj][:],
                                    pt[:, (jj - jj0) * P:(jj - jj0 + 1) * P],
                                    v_sb[:, i, 0:257],
                                    start=(i == 0), stop=(i == j))
                        # epilogue for this view
                        for jj in range(jpc):
                            j = jpc * cqi + jj
                            inv = rms_pool.tile([P, 1], f32, tag="inv")
                            nc.vector.reciprocal(inv[:], ys[jj][:, 256:257])
                            if v == 0:
                                y0 = y0_pool.tile([P, 256], f32, tag="y0")
                                nc.vector.tensor_scalar_mul(
                                    y0[:], ys[jj][:, 0:256], inv[:])
                                y0s.append(y0)
                            else:
                                sc2 = rms_pool.tile([P, 1], f32, tag="inv")
                                nc.vector.tensor_tensor(
                                    sc2[:], inv[:], lam_sb[:], op=OP.mult)
                                yf = qn_pool.tile([P, 256], bf16, tag="yf")
                                nc.vector.scalar_tensor_tensor(
                                    yf[:], ys[jj][:, 0:256], sc2[:], y0s[jj][:],
                                    op0=OP.mult, op1=OP.add)
                                for e in range(2):
                                    tr_y(yt_sb[:, e, j * P:(j + 1) * P],
                                         yf[:, e * P:(e + 1) * P])

                # ================= output projection (partial) =================
                out_eng = nc.scalar if o["out_eng"] == "scalar" else nc.sync
                for tb in range(n_blocks_b):
                    row0 = b * t_dim + tb * P
                    merged = o["osb_merge"] and not (
                        o["tail_split"] and b == b_dim - 1
                        and tb == n_blocks_b - 1)
                    orow = (osb_pool.tile([P, c_dim], f16, tag="orow",
                                          name="orow")
                            if merged else None)
                    for cc in range(c_dim // 512):
                        op_ps = psum_proj.tile([P, 512], f32, tag="pp", name="ops")
                        for e in range(2):
                            nc.tensor.matmul(
                                op_ps[:], yt_sb[:, e, tb * P:(tb + 1) * P],
                                wot_sb[:, e, cc * 512:(cc + 1) * 512],
                                start=(e == 0), stop=(e == 1))
                        osb = (orow[:, cc * 512:(cc + 1) * 512] if merged
                               else osb_pool.tile([P, 512], f16, tag="osb"))
                        oc = o["oproj_copy"]
                        if oc == "alt":
                            oc = "act" if (tb + cc) % 2 == 0 else "dve"
                        if oc == "act":
                            nc.scalar.copy(osb, op_ps[:])
                        else:
                            nc.vector.tensor_copy(osb, op_ps[:])
                        if not merged:
                            out_eng.dma_start(
                                out.ap()[row0:row0 + P,
                                         cc * 512:(cc + 1) * 512], osb)
                    if merged:
                        out_eng.dma_start(
                            out.ap()[row0:row0 + P, :], orow[:])
    nc.compile()
    return nc


_NC_CACHE = {}
TRACE = False        # set True (e.g. from test.py) to capture an NTFF profile
LAST_RESULT = None   # BassKernelResults of the most recent run


def _get_nc(c_dim, t_dim, b_dim):
    key = (c_dim, t_dim, b_dim)
    if key not in _NC_CACHE:
        _NC_CACHE[key] = build_nc(c_dim, t_dim, b_dim)
    return _NC_CACHE[key]


def prep_inputs(x, wq, wk, wv, wo, lq1, lk1, lq2, lk2):
    """Host-side prep: per-core input maps."""
    import ml_dtypes

    bf16 = ml_dtypes.bfloat16
    fp8 = ml_dtypes.float8_e4m3
    b_dim, t_dim, c_dim = x.shape

    lam1 = np.exp(np.sum(lq1.astype(np.float64) * lk1.astype(np.float64)))
    lam2 = np.exp(np.sum(lq2.astype(np.float64) * lk2.astype(np.float64)))
    lam_full = np.float32(lam1 - lam2 + LAMBDA_INIT)

    xt = np.ascontiguousarray(x.reshape(b_dim * t_dim, c_dim).T)
    xh = xt.astype(fp8)
    xl = (xt - xh.astype(np.float32)).astype(fp8)
    xt2 = np.ascontiguousarray(np.stack([xl, xh]))  # planes [lo, hi]
    lamneg = np.full((P, 1), -lam_full, dtype=np.float32)

    in_maps = []
    for h in range(N_CORES):
        sl = slice(h * HEAD_DIM, (h + 1) * HEAD_DIM)
        w64 = np.concatenate([wk[sl].T, wq[sl].T, wv[sl].T],
                             axis=1) * np.float32(WSCALE)
        wh = w64.astype(fp8)
        wl = (w64 - wh.astype(np.float32)).astype(fp8)
        wqkv2 = np.ascontiguousarray(np.stack([wh, wl]))  # planes [hi, lo]
        wot_h = np.ascontiguousarray(
            (wo[:, sl] * ((1.0 - LAMBDA_INIT) / WSCALE)).T).astype(bf16)
        in_maps.append({
            "xt2": xt2, "wqkv2": wqkv2, "wot": wot_h, "lamneg": lamneg,
        })
    return in_maps


_FN_CACHE = {}


def _get_callable(nc):
    """Build (once) a reusable jitted shard_map callable for this module —
    mirrors bass2jax.run_bass_via_pjrt's multi-core path, but cached so
    repeat kernel() calls skip retracing."""
    if id(nc) in _FN_CACHE:
        return _FN_CACHE[id(nc)]
    import jax
    from jax.sharding import Mesh, PartitionSpec, NamedSharding
    from jax.experimental.shard_map import shard_map
    import concourse.mybir as mybir
    import concourse.bass2jax as b2j

    b2j.install_neuronx_cc_hook()
    pname = nc.partition_id_tensor.name if nc.partition_id_tensor else None
    in_names, out_names, out_avals, zero_shapes = [], [], [], []
    for alloc in nc.m.functions[0].allocations:
        if not isinstance(alloc, mybir.MemoryLocationSet):
            continue
        name = alloc.memorylocations[0].name
        if alloc.kind == "ExternalInput":
            if name != pname:
                in_names.append(name)
        elif alloc.kind == "ExternalOutput":
            out_names.append(name)
            shape = tuple(alloc.tensor_shape)
            dtype = mybir.dt.np(alloc.dtype)
            out_avals.append(jax.core.ShapedArray(shape, dtype))
            zero_shapes.append((shape, dtype))
    n_params = len(in_names)
    all_in = in_names + out_names
    if pname is not None:
        all_in = all_in + [pname]

    def _body(*args):
        operands = list(args)
        if pname is not None:
            operands.append(b2j.partition_id_tensor())
        return tuple(b2j._bass_exec_p.bind(
            *operands,
            out_avals=tuple(out_avals),
            in_names=tuple(all_in),
            out_names=tuple(out_names),
            lowering_input_output_aliases=(),
            sim_require_finite=True,
            sim_require_nnan=True,
            nc=nc,
        ))

    devices = jax.devices()[:N_CORES]
    mesh = Mesh(np.asarray(devices), ("core",))
    nio = n_params + len(out_names)
    fn = jax.jit(shard_map(_body, mesh=mesh,
                           in_specs=(PartitionSpec("core"),) * nio,
                           out_specs=(PartitionSpec("core"),) * len(out_names),
                           check_rep=False),
                 donate_argnums=tuple(range(n_params, nio)), keep_unused=True)
    sh = NamedSharding(mesh, PartitionSpec("core"))
    entry = (fn, in_names, out_names, zero_shapes, sh)
    _FN_CACHE[id(nc)] = entry
    return entry


def kernel(x, wq, wk, wv, wo, lq1, lk1, lq2, lk2):
    b_dim, t_dim, c_dim = x.shape
    in_maps = prep_inputs(x, wq, wk, wv, wo, lq1, lk1, lq2, lk2)
    nc = _get_nc(c_dim, t_dim, b_dim)

    try:
        import jax
        fn, in_names, out_names, zero_shapes, sh = _get_callable(nc)
        concat_in = [
            np.concatenate([np.asarray(in_maps[c][n]) for c in range(N_CORES)],
                           axis=0) for n in in_names]
        concat_zeros = [np.zeros((N_CORES * s[0], *s[1:]), d)
                        for s, d in zero_shapes]
        dev_in = [jax.device_put(a, sh) for a in concat_in]
        dev_zero = [jax.device_put(a, sh) for a in concat_zeros]
        outs = fn(*dev_in, *dev_zero)
        arr = np.asarray(outs[out_names.index("out")])
        acc = arr.reshape(N_CORES, b_dim * t_dim, c_dim).astype(
            np.float32).sum(axis=0)
    except Exception:
        from concourse.bass_utils import run_bass_kernel_spmd
        res = run_bass_kernel_spmd(nc, in_maps, list(range(N_CORES)),
                                   trace=TRACE)
        global LAST_RESULT
        LAST_RESULT = res
        acc = np.zeros((b_dim * t_dim, c_dim), dtype=np.float32)
        for h in range(N_CORES):
            acc += res.results[h]["out"].astype(np.float32)
    return acc.reshape(b_dim, t_dim, c_dim)


# revision 12
# speedup vs baseline: 1.0755x; 1.0633x over previous
"""Differential attention (B=2, T=2048, C=2048, 8 heads x 256) on 8 trn2 cores.

Sharding: tensor-parallel over the 8 effective heads — core h computes head h's
projections + attention and a partial output projection; host sums partials.

Projections run in fp8e4m3 with a 3-term residual (hi/lo planes of both x and
w, DoubleRow matmuls): (xh+xl)@(wh+wl) ~ xh@wh + (xh@wl + xl@wh), each pair of
128-contraction tiles fused into one DoubleRow instruction at 0.5 cycles/row —
25% fewer PE cycles than bf16 at bf16-level accuracy. Weights are host-scaled
by 64 into fp8's normal range; q/k absorb the scale in rms-norm (the Newton
rsqrt seed is refit for the 4096x mean-square), v stays 64x and the out-proj
weights carry 1/64.

Attention (scores S.T = K.T@Q -> exp -> P.T @ V with a ones-column giving the
softmax denominator) stays bf16: P = exp(s) spans e^-inf..e^11 which fp8
cannot represent, and bf16 q/k are needed for exp accuracy. Causal blocks
skipped; diagonal blocks masked multiplicatively post-exp.

Output is stored fp16 (halves store DMA); host sums the 8 partials in f32.
"""

import math
from contextlib import ExitStack

import numpy as np

# ---- problem constants (hardcoded per the harness contract) ----
B = 2
T = 2048
C = 2048
N_HEAD = 8
HEAD_DIM = 256
HALF = 128
LAMBDA_INIT = 0.8
RMS_EPS = 1.1920929e-07
N_CORES = 8

P = 128          # partitions
TOK_CHUNK = 512  # projection tok chunk (DMA granularity)
WSCALE = 64.0    # host-side weight scale into fp8 normal range
MSCALE = WSCALE * WSCALE  # mean-square scale (4096)

# Newton rsqrt seed: least-squares quadratic fit of rsqrt on m in
# MSCALE*[0.3, 2.0] (q/k mean-squares after the 64x weight scale), clamped.
RSQ_A = 2.07556761 / WSCALE
RSQ_B = -1.47991565 / (MSCALE * WSCALE)
RSQ_C = 0.41306651 / (MSCALE * MSCALE * WSCALE)
RSQ_CLAMP = 0.05 / WSCALE

DEFAULT_OPTS = dict(
    att_chunk=256,       # attention q-chunk width (256 or 512)
    oproj_copy="alt",    # out-proj PSUM->SBUF evacuation: "act"|"dve"|"alt"
    psum=(3, 3, 2),      # banks: (proj, st, y) — must sum to <= 8
    pt_bufs=5,           # P.T tile double-buffer depth
    xc_bufs=2,           # x chunk prefetch depth
    vcopy="dve",         # "act" | "dve": V PSUM->SBUF copy engine
    osb_merge=True,      # one output-store DMA per tok block (vs per c-chunk)
    narrow_top=True,     # compute only the valid half of the top causal row
    tr_pool="st",        # "st" | "pp": PSUM pool used by PE transposes
    rms_bufs=4,
    qn_bufs=6,
    y0_mult=2,
    ksq_eng="dve",       # "act" | "dve": engine computing k^2
    ktcopy_eng="dve",    # "act" | "dve": engine evacuating KT psum
    ytr_pool="y",        # "st" | "y": PSUM pool for the y transposes
    ksq_src="sbuf",      # "psum" | "sbuf": k^2 input
    chunk_order="asc",   # "asc" | "desc": attention q-chunk processing order
    out_eng="scalar",    # "scalar" | "sync": queue for output stores
    tail_split=False,    # stream the final block's stores per c-chunk
)


def build_nc(c_dim, t_dim, b_dim, **opts):
    """Build the per-core Bass module. All shapes in tokens/channels."""
    import concourse.mybir as mybir
    import concourse.tile as tile
    from concourse import bacc
    from concourse.masks import make_identity, make_upper_triangular

    o = dict(DEFAULT_OPTS)
    o.update(opts)
    QCH = o["att_chunk"]
    jpc = QCH // P  # j-blocks per attention chunk

    dt = mybir.dt
    f32 = dt.float32
    f16 = dt.float16
    bf16 = dt.bfloat16
    fp8 = dt.float8e4
    AF = mybir.ActivationFunctionType
    OP = mybir.AluOpType
    DR = mybir.MatmulPerfMode.DoubleRow

    n_ctiles = c_dim // P            # contraction tiles over C
    npairs = n_ctiles // 2
    ntok = b_dim * t_dim             # total token rows
    n_blocks_b = t_dim // P          # 128-tok blocks per batch
    n_qchunks = t_dim // QCH         # attention q chunks per batch
    blocks_per_chunk = TOK_CHUNK // P
    inv_sqrt_half = 1.0 / math.sqrt(HALF)
    VP = 272                         # V tile pitch (256 vals + 1 ones + pad)

    nc = bacc.Bacc()
    # x planes: [lo, hi]; w planes: [hi, lo] — cross-term DoubleRow pairs
    # (x_lo*w_hi + x_hi*w_lo) then use natural ascending slices on both.
    xt2 = nc.declare_dram_parameter("xt2", [2, c_dim, ntok], fp8, isOutput=False)
    # ramp tensors: host-prepacked partition-major (fully contiguous per
    # partition row) so the DMA model sees >=512B descriptors at full speed.
    # xm0..3: chunk-0 x in 128-tok pieces; wk2p: k cols both planes; wqv
    # plane-split.
    xms = [nc.declare_dram_parameter(f"xm{t}", [P, 2 * n_ctiles * P], fp8,
                                     isOutput=False)
           for t in range(blocks_per_chunk)]
    wk2p = nc.declare_dram_parameter("wk2p", [P, 2 * n_ctiles * 256], fp8,
                                     isOutput=False)
    wqvh = nc.declare_dram_parameter("wqvh", [P, n_ctiles * 512], fp8,
                                     isOutput=False)
    wqvl = nc.declare_dram_parameter("wqvl", [P, n_ctiles * 512], fp8,
                                     isOutput=False)
    wot = nc.declare_dram_parameter("wot", [HEAD_DIM, c_dim], bf16, isOutput=False)
    lamneg = nc.declare_dram_parameter("lamneg", [P, 1], f32, isOutput=False)
    out = nc.declare_dram_parameter("out", [ntok, c_dim], f16, isOutput=True)

    xt_r = xt2.ap().rearrange("v (i p) t -> p v i t", p=P)    # [128,2,nct,ntok]
    wot_r = wot.ap().rearrange("(e p) n -> p e n", p=P)       # [128, 2, c_dim]

    with tile.TileContext(nc) as tc:
        with ExitStack() as ctx:
            # ---- persistent SBUF ----
            const_pool = ctx.enter_context(tc.tile_pool(name="const", bufs=1))
            wk_sb = const_pool.tile([P, 2, n_ctiles, 256], fp8, name="wk_sb")
            wqv_sb = const_pool.tile([P, 2, n_ctiles, 512], fp8, name="wqv_sb")
            wot_sb = const_pool.tile([P, 2, c_dim], bf16, name="wot_sb")
            lam_sb = const_pool.tile([P, 1], f32, name="lam_sb")
            ident = const_pool.tile([P, P], bf16, name="ident")
            trimask = const_pool.tile([P, P], bf16, name="trimask")
            ones_sb = const_pool.tile([P, 1], bf16, name="ones_sb")
            nc.vector.memset(ones_sb[:], 1.0)

            xc_pool = ctx.enter_context(tc.tile_pool(name="xc", bufs=o["xc_bufs"]))

            # ---- ramp: chunk 0 comes in as 4 part-major 128-tok minis,
            # interleaved with the weight loads, so PE starts ~7us in and is
            # fed continuously. All transfers have >=512B descriptors.
            xm_sb = [const_pool.tile([P, 2, n_ctiles, P], fp8, name=f"xm{t}")
                     for t in range(blocks_per_chunk)]
            nc.sync.dma_start(xm_sb[0][:],
                              xms[0].ap().rearrange("p (v i n) -> p v i n",
                                                    v=2, i=n_ctiles))
            nc.sync.dma_start(wk_sb[:],
                              wk2p.ap().rearrange("p (v i n) -> p v i n",
                                                  v=2, i=n_ctiles))
            nc.sync.dma_start(xm_sb[1][:],
                              xms[1].ap().rearrange("p (v i n) -> p v i n",
                                                    v=2, i=n_ctiles))
            nc.sync.dma_start(wqv_sb[:, 0],
                              wqvh.ap().rearrange("p (i n) -> p i n",
                                                  i=n_ctiles))
            nc.sync.dma_start(xm_sb[2][:],
                              xms[2].ap().rearrange("p (v i n) -> p v i n",
                                                    v=2, i=n_ctiles))
            nc.sync.dma_start(wqv_sb[:, 1],
                              wqvl.ap().rearrange("p (i n) -> p i n",
                                                  i=n_ctiles))
            nc.sync.dma_start(xm_sb[3][:],
                              xms[3].ap().rearrange("p (v i n) -> p v i n",
                                                    v=2, i=n_ctiles))
            nc.scalar.dma_start(wot_sb[:], wot_r[:])
            nc.scalar.dma_start(lam_sb[:], lamneg.ap())
            make_identity(nc, ident[:])
            # 1.0 where kk <= q (partition <= free), else 0
            make_upper_triangular(nc, trimask[:], val=1.0, diag=True)

            qt_pool = ctx.enter_context(tc.tile_pool(name="qt", bufs=2))
            kt_pool = ctx.enter_context(tc.tile_pool(name="kt", bufs=2))
            ksq_pool = ctx.enter_context(tc.tile_pool(name="ksq", bufs=2))
            kscale_pool = ctx.enter_context(tc.tile_pool(name="kscale", bufs=2))
            v_pool = ctx.enter_context(tc.tile_pool(name="v", bufs=2))
            yt_pool = ctx.enter_context(tc.tile_pool(name="yt", bufs=2))
            pt_pool = ctx.enter_context(tc.tile_pool(name="pt", bufs=o["pt_bufs"]))
            y0_pool = ctx.enter_context(tc.tile_pool(name="y0", bufs=o["y0_mult"] * jpc))
            osb_pool = ctx.enter_context(tc.tile_pool(name="osb", bufs=3))
            qn_pool = ctx.enter_context(tc.tile_pool(name="qn", bufs=o["qn_bufs"]))
            sq_pool = ctx.enter_context(tc.tile_pool(name="sq", bufs=2))
            rms_pool = ctx.enter_context(tc.tile_pool(name="rms", bufs=o["rms_bufs"]))
            nproj, nst, ny = o["psum"]
            psum_proj = ctx.enter_context(
                tc.tile_pool(name="psum_proj", bufs=nproj, space="PSUM"))
            psum_st = ctx.enter_context(
                tc.tile_pool(name="psum_st", bufs=nst, space="PSUM"))
            psum_y = ctx.enter_context(
                tc.tile_pool(name="psum_y", bufs=ny, space="PSUM"))

            tr_psum = psum_st if o["tr_pool"] == "st" else psum_proj
            tr_tag = o["tr_pool"] if o["tr_pool"] == "st" else "pp"
            tr_shape = QCH if o["tr_pool"] == "st" else 512

            def pe_transpose(dst_ap, src_ap):
                trp = tr_psum.tile([P, tr_shape], bf16, tag=tr_tag,
                                   name="trp")[:, :P]
                nc.tensor.transpose(trp, src_ap, ident[:])
                nc.vector.tensor_copy(dst_ap, trp)

            def pe_transpose_y(dst_ap, src_ap):
                trp = psum_y.tile([P, 257], bf16, tag="y", name="trpy")[:, :P]
                nc.tensor.transpose(trp, src_ap, ident[:])
                nc.vector.tensor_copy(dst_ap, trp)

            tr_qk = pe_transpose
            tr_y = pe_transpose_y if o["ytr_pool"] == "y" else pe_transpose

            def dr_proj(out_ap, x_ap, w_ap, x_stationary):
                """3-term residual fp8 accumulation into out_ap [128, N].

                x_ap(pl, i, sl): plane/ctile/slice accessor; same for w_ap.
                Planes: x [lo, hi], w [hi, lo]. Emits 1.5*npairs DoubleRow
                matmuls; caller's region gets start on the first, stop on the
                last.
                """
                calls = []
                for pr in range(npairs):  # hi @ hi, k-tile pairs
                    calls.append((x_ap(1, slice(2 * pr, 2 * pr + 2)),
                                  w_ap(0, slice(2 * pr, 2 * pr + 2))))
                for i in range(n_ctiles):  # x_lo@w_hi + x_hi@w_lo per tile
                    calls.append((x_ap(slice(0, 2), i),
                                  w_ap(slice(0, 2), i)))
                n = len(calls)
                for idx, (xs, ws) in enumerate(calls):
                    lhsT, rhs = (xs, ws) if x_stationary else (ws, xs)
                    nc.tensor.matmul(out_ap, lhsT, rhs,
                                     start=(idx == 0), stop=(idx == n - 1),
                                     perf_mode=DR)

            def rsqrt_newton(dst, m, tmp_pool, width):
                """DVE-only rsqrt on the MSCALE-shifted mean-square range.
                Returns the tile holding the result (may be a fresh tile)."""
                t1 = tmp_pool.tile([P, width], f32, tag="rms", name="rsq_t1")
                nc.vector.tensor_tensor(t1[:], m, m, op=OP.mult)
                nc.vector.tensor_scalar(dst, m, RSQ_B, RSQ_A, OP.mult, OP.add)
                nc.vector.scalar_tensor_tensor(dst, t1[:], RSQ_C, dst,
                                               op0=OP.mult, op1=OP.add)
                nc.vector.tensor_scalar_max(dst, dst, RSQ_CLAMP)
                for _ in range(2):
                    nc.vector.tensor_tensor(t1[:], dst, dst, op=OP.mult)
                    nc.vector.scalar_tensor_tensor(t1[:], t1[:], -0.5, m,
                                                   op0=OP.mult, op1=OP.mult)
                    nc.vector.tensor_scalar(t1[:], t1[:], 1.0, 1.5,
                                            OP.mult, OP.add)
                    nc.vector.tensor_tensor(dst, dst, t1[:], op=OP.mult)

            for b in range(b_dim):
                qt_sb = qt_pool.tile([P, 2, t_dim], bf16, name=f"qt_b{b}", tag="qt")
                kt_sb = kt_pool.tile([P, 2, t_dim], bf16, name=f"kt_b{b}", tag="kt")
                v_sb = v_pool.tile([P, n_blocks_b, VP], bf16, name=f"v_b{b}", tag="v")
                yt_sb = yt_pool.tile([P, 2, t_dim], bf16, name=f"yt_b{b}", tag="yt")
                kscale_sb = kscale_pool.tile([P, 2 * n_blocks_b], f32,
                                             name=f"ksc_b{b}", tag="ksc")

                # ================= projections =================
                for ch in range(t_dim // TOK_CHUNK):
                    tok0 = b * t_dim + ch * TOK_CHUNK
                    first_ch = (b == 0 and ch == 0)
                    if first_ch:
                        xc = None
                    else:
                        xc = xc_pool.tile([P, 2, n_ctiles, TOK_CHUNK], fp8,
                                          tag="xc")
                        nc.sync.dma_start(xc[:], xt_r[:, :, :, tok0:tok0 + TOK_CHUNK])

                    # --- K projection straight into [d, tok] layout ---
                    # chunk 0 in 128-tok pieces (one per ramp mini)
                    kpieces = ([(t * P, P, xm_sb[t], 0)
                                for t in range(blocks_per_chunk)] if first_ch
                               else [(h2 * 256, 256, xc, h2 * 256)
                                     for h2 in range(TOK_CHUNK // 256)])
                    kssq = psum_st.tile([P, QCH], f32, tag="st",
                                        name="kssq")[:, :8]
                    for v in range(2):
                        ktp = psum_proj.tile([P, 512], f32, tag="pp", name="ktp")
                        for off, wdt, src_t, soff in kpieces:
                            dr_proj(
                                ktp[:, off:off + wdt],
                                lambda pl, i, src_t=src_t, soff=soff, wdt=wdt:
                                    src_t[:, pl, i, soff:soff + wdt],
                                lambda pl, i, v=v: wk_sb[:, pl, i,
                                                         v * P:(v + 1) * P],
                                x_stationary=False)
                        ktdst = kt_sb[:, v, ch * TOK_CHUNK:(ch + 1) * TOK_CHUNK]
                        if o["ktcopy_eng"] == "act":
                            nc.scalar.copy(ktdst, ktp[:])
                        else:
                            nc.vector.tensor_copy(ktdst, ktp[:])
                        ksq = ksq_pool.tile([P, TOK_CHUNK], bf16, tag="ksq")
                        ksrc = ktdst if o["ksq_src"] == "sbuf" else ktp[:]
                        if o["ksq_eng"] == "act":
                            nc.scalar.activation(ksq[:], ksrc, AF.Square)
                        else:
                            nc.vector.tensor_tensor(ksq[:], ksrc, ksrc,
                                                    op=OP.mult)
                        for t in range(blocks_per_chunk):
                            nc.tensor.matmul(
                                kssq[:, 2 * t + v:2 * t + v + 1],
                                ksq[:, t * P:(t + 1) * P], ones_sb[:],
                                start=True, stop=True)
                    # kscale = (1/64)/sqrt(mean(k^2) + eps) * inv_sqrt_half
                    ksl = kscale_sb[:, ch * 2 * blocks_per_chunk:
                                    (ch + 1) * 2 * blocks_per_chunk]
                    km = rms_pool.tile([P, 8], f32, tag="rms", name="km")
                    nc.vector.tensor_scalar(km[:], kssq[:], 1.0 / HALF,
                                            MSCALE * RMS_EPS, OP.mult, OP.add)
                    rsqrt_newton(ksl, km[:], rms_pool, 8)
                    nc.vector.tensor_scalar_mul(ksl, ksl, inv_sqrt_half)

                    for tl in range(blocks_per_chunk):
                        tb = ch * blocks_per_chunk + tl
                        # one bank: [q1 q2 | v]
                        qv = psum_proj.tile([P, 512], f32, tag="pp", name="qv")
                        xsrc = xm_sb[tl] if first_ch else xc
                        tsl = (slice(0, P) if first_ch
                               else slice(tl * P, (tl + 1) * P))
                        for h2 in range(2):
                            dr_proj(
                                qv[:, h2 * 256:(h2 + 1) * 256],
                                lambda pl, i, xsrc=xsrc, tsl=tsl:
                                    xsrc[:, pl, i, tsl],
                                lambda pl, i, h2=h2: wqv_sb[
                                    :, pl, i, h2 * 256:(h2 + 1) * 256],
                                x_stationary=True)
                        halves = [qv[:, 0:128], qv[:, 128:256]]
                        rmsg = rms_pool.tile([P, 2], f32, tag="rms")
                        for j, h in enumerate(halves):
                            sq = sq_pool.tile([P, P], bf16, tag="sq", name="sq")
                            nc.scalar.activation(
                                sq[:], h, AF.Square,
                                accum_out=rmsg[:, j:j + 1])
                        nc.vector.tensor_scalar(rmsg[:], rmsg[:], 1.0 / HALF,
                                                MSCALE * RMS_EPS, OP.mult, OP.add)
                        yv = rms_pool.tile([P, 2], f32, tag="rms")
                        rsqrt_newton(yv[:], rmsg[:], rms_pool, 2)
                        for j, h in enumerate(halves):
                            qn = qn_pool.tile([P, P], bf16, tag="qn")
                            nc.vector.tensor_scalar_mul(qn[:], h, yv[:, j:j + 1])
                            tr_qk(qt_sb[:, j, tb * P:(tb + 1) * P], qn[:])
                        # V (+ ones column for the softmax denominator)
                        vsrc = qv[:, 256:512]
                        if o["vcopy"] == "act":
                            nc.scalar.copy(v_sb[:, tb, 0:256], vsrc)
                        else:
                            nc.vector.tensor_copy(v_sb[:, tb, 0:256], vsrc)
                        nc.vector.memset(v_sb[:, tb, 256:257], 1.0)

                # ================= attention =================
                cqi_order = (range(n_qchunks) if o["chunk_order"] == "asc"
                             else range(n_qchunks - 1, -1, -1))
                for cqi in cqi_order:
                    q0 = cqi * QCH
                    jmax = jpc * cqi + (jpc - 1)   # top kk-tile in this chunk
                    y0s = []
                    for v in range(2):
                        ys = [psum_y.tile([P, 257], f32, tag="y", name="ys")
                              for _ in range(jpc)]
                        for i in range(jmax + 1):
                            # jj0: first valid j-slot for this row (causal)
                            jj0 = max(0, i - jpc * cqi) if o["narrow_top"] else 0
                            w = QCH - jj0 * P
                            st = psum_st.tile([P, QCH], f32, tag="st",
                                              name="st")[:, :w]
                            nc.tensor.matmul(
                                st[:], kt_sb[:, v, i * P:(i + 1) * P],
                                qt_sb[:, v, q0 + jj0 * P:q0 + QCH],
                                start=True, stop=True)
                            pt = pt_pool.tile([P, QCH], bf16, tag="pt", name="pt")[:, :w]
                            nc.scalar.activation(
                                pt[:], st[:], AF.Exp,
                                scale=kscale_sb[:, 2 * i + v:2 * i + v + 1])
                            dj = i - jpc * cqi    # diagonal j-slot if >= 0
                            if dj >= 0:
                                nc.vector.tensor_tensor(
                                    pt[:, (dj - jj0) * P:(dj - jj0 + 1) * P],
                                    pt[:, (dj - jj0) * P:(dj - jj0 + 1) * P],
                                    trimask[:], op=OP.mult)
                            for jj in range(jj0, jpc):
                                j = jpc * cqi + jj
                                if i > j:
                                    continue
                                nc.tensor.matmul(
                                    ys[jj][:],
                                    pt[:, (jj - jj0) * P:(jj - jj0 + 1) * P],
                                    v_sb[:, i, 0:257],
                                    start=(i == 0), stop=(i == j))
                        # epilogue for this view
                        for jj in range(jpc):
                            j = jpc * cqi + jj
                            inv = rms_pool.tile([P, 1], f32, tag="inv")
                            nc.vector.reciprocal(inv[:], ys[jj][:, 256:257])
                            if v == 0:
                                y0 = y0_pool.tile([P, 256], f32, tag="y0")
                                nc.vector.tensor_scalar_mul(
                                    y0[:], ys[jj][:, 0:256], inv[:])
                                y0s.append(y0)
                            else:
                                sc2 = rms_pool.tile([P, 1], f32, tag="inv")
                                nc.vector.tensor_tensor(
                                    sc2[:], inv[:], lam_sb[:], op=OP.mult)
                                yf = qn_pool.tile([P, 256], bf16, tag="yf")
                                nc.vector.scalar_tensor_tensor(
                                    yf[:], ys[jj][:, 0:256], sc2[:], y0s[jj][:],
                                    op0=OP.mult, op1=OP.add)
                                for e in range(2):
                                    tr_y(yt_sb[:, e, j * P:(j + 1) * P],
                                         yf[:, e * P:(e + 1) * P])

                # ================= output projection (partial) =================
                out_eng = nc.scalar if o["out_eng"] == "scalar" else nc.sync
                for tb in range(n_blocks_b):
                    row0 = b * t_dim + tb * P
                    merged = o["osb_merge"] and not (
                        o["tail_split"] and b == b_dim - 1
                        and tb == n_blocks_b - 1)
                    orow = (osb_pool.tile([P, c_dim], f16, tag="orow",
                                          name="orow")
                            if merged else None)
                    for cc in range(c_dim // 512):
                        op_ps = psum_proj.tile([P, 512], f32, tag="pp", name="ops")
                        for e in range(2):
                            nc.tensor.matmul(
                                op_ps[:], yt_sb[:, e, tb * P:(tb + 1) * P],
                                wot_sb[:, e, cc * 512:(cc + 1) * 512],
                                start=(e == 0), stop=(e == 1))
                        osb = (orow[:, cc * 512:(cc + 1) * 512] if merged
                               else osb_pool.tile([P, 512], f16, tag="osb"))
                        oc = o["oproj_copy"]
                        if oc == "alt":
                            oc = "act" if (tb + cc) % 2 == 0 else "dve"
                        if oc == "act":
                            nc.scalar.copy(osb, op_ps[:])
                        else:
                            nc.vector.tensor_copy(osb, op_ps[:])
                        if not merged:
                            out_eng.dma_start(
                                out.ap()[row0:row0 + P,
                                         cc * 512:(cc + 1) * 512], osb)
                    if merged:
                        out_eng.dma_start(
                            out.ap()[row0:row0 + P, :], orow[:])
    nc.compile()
    return nc


_NC_CACHE = {}
TRACE = False        # set True (e.g. from test.py) to capture an NTFF profile
LAST_RESULT = None   # BassKernelResults of the most recent run


def _get_nc(c_dim, t_dim, b_dim):
    key = (c_dim, t_dim, b_dim)
    if key not in _NC_CACHE:
        _NC_CACHE[key] = build_nc(c_dim, t_dim, b_dim)
    return _NC_CACHE[key]


def prep_inputs(x, wq, wk, wv, wo, lq1, lk1, lq2, lk2):
    """Host-side prep: per-core input maps."""
    import ml_dtypes

    bf16 = ml_dtypes.bfloat16
    fp8 = ml_dtypes.float8_e4m3
    b_dim, t_dim, c_dim = x.shape

    lam1 = np.exp(np.sum(lq1.astype(np.float64) * lk1.astype(np.float64)))
    lam2 = np.exp(np.sum(lq2.astype(np.float64) * lk2.astype(np.float64)))
    lam_full = np.float32(lam1 - lam2 + LAMBDA_INIT)

    n_ctiles = c_dim // P
    xt = np.ascontiguousarray(x.reshape(b_dim * t_dim, c_dim).T)
    xh = xt.astype(fp8)
    xl = (xt - xh.astype(np.float32)).astype(fp8)
    xt2 = np.ascontiguousarray(np.stack([xl, xh]))  # planes [lo, hi]
    lamneg = np.full((P, 1), -lam_full, dtype=np.float32)

    def part_major(a):
        """[..., 2?, C, n] -> [128, prod(rest)] with c = i*128 + p."""
        if a.ndim == 2:
            a = a[None]
        v, cdim, n = a.shape
        a = a.reshape(v, n_ctiles, P, n).transpose(2, 0, 1, 3)
        return np.ascontiguousarray(a.reshape(P, v * n_ctiles * n))

    xm = {f"xm{t}": part_major(xt2[:, :, t * P:(t + 1) * P])
          for t in range(TOK_CHUNK // P)}

    in_maps = []
    for h in range(N_CORES):
        sl = slice(h * HEAD_DIM, (h + 1) * HEAD_DIM)
        wk64 = wk[sl].T * np.float32(WSCALE)
        wkh = wk64.astype(fp8)
        wkl = (wk64 - wkh.astype(np.float32)).astype(fp8)
        wk2p = part_major(np.stack([wkh, wkl]))  # planes [hi, lo]
        wqv64 = np.concatenate([wq[sl].T, wv[sl].T], axis=1) * np.float32(WSCALE)
        wqvh_ = wqv64.astype(fp8)
        wqvl_ = (wqv64 - wqvh_.astype(np.float32)).astype(fp8)
        wot_h = np.ascontiguousarray(
            (wo[:, sl] * ((1.0 - LAMBDA_INIT) / WSCALE)).T).astype(bf16)
        in_maps.append({
            "xt2": xt2, "wk2p": wk2p,
            "wqvh": part_major(wqvh_), "wqvl": part_major(wqvl_),
            "wot": wot_h, "lamneg": lamneg, **xm,
        })
    return in_maps


_FN_CACHE = {}


def _get_callable(nc):
    """Build (once) a reusable jitted shard_map callable for this module —
    mirrors bass2jax.run_bass_via_pjrt's multi-core path, but cached so
    repeat kernel() calls skip retracing."""
    if id(nc) in _FN_CACHE:
        return _FN_CACHE[id(nc)]
    import jax
    from jax.sharding import Mesh, PartitionSpec, NamedSharding
    from jax.experimental.shard_map import shard_map
    import concourse.mybir as mybir
    import concourse.bass2jax as b2j

    b2j.install_neuronx_cc_hook()
    pname = nc.partition_id_tensor.name if nc.partition_id_tensor else None
    in_names, out_names, out_avals, zero_shapes = [], [], [], []
    for alloc in nc.m.functions[0].allocations:
        if not isinstance(alloc, mybir.MemoryLocationSet):
            continue
        name = alloc.memorylocations[0].name
        if alloc.kind == "ExternalInput":
            if name != pname:
                in_names.append(name)
        elif alloc.kind == "ExternalOutput":
            out_names.append(name)
            shape = tuple(alloc.tensor_shape)
            dtype = mybir.dt.np(alloc.dtype)
            out_avals.append(jax.core.ShapedArray(shape, dtype))
            zero_shapes.append((shape, dtype))
    n_params = len(in_names)
    all_in = in_names + out_names
    if pname is not None:
        all_in = all_in + [pname]

    def _body(*args):
        operands = list(args)
        if pname is not None:
            operands.append(b2j.partition_id_tensor())
        return tuple(b2j._bass_exec_p.bind(
            *operands,
            out_avals=tuple(out_avals),
            in_names=tuple(all_in),
            out_names=tuple(out_names),
            lowering_input_output_aliases=(),
            sim_require_finite=True,
            sim_require_nnan=True,
            nc=nc,
        ))

    devices = jax.devices()[:N_CORES]
    mesh = Mesh(np.asarray(devices), ("core",))
    nio = n_params + len(out_names)
    fn = jax.jit(shard_map(_body, mesh=mesh,
                           in_specs=(PartitionSpec("core"),) * nio,
                           out_specs=(PartitionSpec("core"),) * len(out_names),
                           check_rep=False),
                 donate_argnums=tuple(range(n_params, nio)), keep_unused=True)
    sh = NamedSharding(mesh, PartitionSpec("core"))
    entry = (fn, in_names, out_names, zero_shapes, sh)
    _FN_CACHE[id(nc)] = entry
    return entry


def kernel(x, wq, wk, wv, wo, lq1, lk1, lq2, lk2):
    b_dim, t_dim, c_dim = x.shape
    in_maps = prep_inputs(x, wq, wk, wv, wo, lq1, lk1, lq2, lk2)
    nc = _get_nc(c_dim, t_dim, b_dim)

    try:
        import jax
        fn, in_names, out_names, zero_shapes, sh = _get_callable(nc)
        concat_in = [
            np.concatenate([np.asarray(in_maps[c][n]) for c in range(N_CORES)],
                           axis=0) for n in in_names]
        concat_zeros = [np.zeros((N_CORES * s[0], *s[1:]), d)
                        for s, d in zero_shapes]
        dev_in = [jax.device_put(a, sh) for a in concat_in]
        dev_zero = [jax.device_put(a, sh) for a in concat_zeros]
        outs = fn(*dev_in, *dev_zero)
        arr = np.asarray(outs[out_names.index("out")])
        acc = arr.reshape(N_CORES, b_dim * t_dim, c_dim).astype(
            np.float32).sum(axis=0)
    except Exception:
        from concourse.bass_utils import run_bass_kernel_spmd
        res = run_bass_kernel_spmd(nc, in_maps, list(range(N_CORES)),
                                   trace=TRACE)
        global LAST_RESULT
        LAST_RESULT = res
        acc = np.zeros((b_dim * t_dim, c_dim), dtype=np.float32)
        for h in range(N_CORES):
            acc += res.results[h]["out"].astype(np.float32)
    return acc.reshape(b_dim, t_dim, c_dim)


# revision 33
# speedup vs baseline: 1.1921x; 1.1084x over previous
"""Differential attention (B=2, T=2048, C=2048, 8 heads x 256) on 8 trn2 cores.

Sharding: tensor-parallel over the 8 effective heads — core h computes head h's
projections + attention and a partial output projection; host sums partials.

Projections run in fp8e4m3 with a 3-term residual (hi/lo planes of both x and
w, DoubleRow matmuls): (xh+xl)@(wh+wl) ~ xh@wh + (xh@wl + xl@wh), each pair of
128-contraction tiles fused into one DoubleRow instruction at 0.5 cycles/row —
25% fewer PE cycles than bf16 at bf16-level accuracy. Weights are host-scaled
by 64 into fp8's normal range; q/k absorb the scale in rms-norm (the Newton
rsqrt seed is refit for the 4096x mean-square), v stays 64x and the out-proj
weights carry 1/64.

Attention (scores S.T = K.T@Q -> exp -> P.T @ V with a ones-column giving the
softmax denominator) stays bf16: P = exp(s) spans e^-inf..e^11 which fp8
cannot represent, and bf16 q/k are needed for exp accuracy. Causal blocks
skipped; diagonal blocks masked multiplicatively post-exp.

Output is stored fp16 (halves store DMA); host sums the 8 partials in f32.
"""

import math
from contextlib import ExitStack

import numpy as np

# ---- problem constants (hardcoded per the harness contract) ----
B = 2
T = 2048
C = 2048
N_HEAD = 8
HEAD_DIM = 256
HALF = 128
LAMBDA_INIT = 0.8
RMS_EPS = 1.1920929e-07
N_CORES = 8

P = 128          # partitions
TOK_CHUNK = 512  # projection tok chunk (DMA granularity)
WSCALE = 64.0    # host-side weight scale into fp8 normal range
MSCALE = WSCALE * WSCALE  # mean-square scale (4096)

# Newton rsqrt seed: least-squares quadratic fit of rsqrt on m in
# MSCALE*[0.3, 2.0] (q/k mean-squares after the 64x weight scale), clamped.
RSQ_A = 2.07556761 / WSCALE
RSQ_B = -1.47991565 / (MSCALE * WSCALE)
RSQ_C = 0.41306651 / (MSCALE * MSCALE * WSCALE)
RSQ_CLAMP = 0.05 / WSCALE

DEFAULT_OPTS = dict(
    att_chunk=256,       # attention q-chunk width (256 or 512)
    oproj_copy="dve",    # out-proj PSUM->SBUF evac: act|dve|alt (PSUM: no pool)
    trimask_eng="dve",   # "dve" | "pool": diagonal-block mask multiply
    psum=(3, 3, 2),      # banks: (proj, st, y) — must sum to <= 8
    pt_bufs=12,          # P.T tile double-buffer depth
    xc_bufs=2,           # x chunk prefetch depth
    vcopy="dve",         # "act" | "dve": V PSUM->SBUF copy engine (PSUM: no pool)
    osb_merge=True,      # one output-store DMA per tok block (vs per c-chunk)
    narrow_top=True,     # compute only the valid half of the top causal row
    tr_pool="st",        # "st" | "pp": PSUM pool used by PE transposes
    rms_bufs=4,
    qn_bufs=6,
    y0_mult=2,
    ksq_eng="dve",       # "act" | "dve": engine computing k^2
    ktcopy_eng="act",    # "act" | "dve": engine evacuating KT psum
    ytr_pool="y",        # "st" | "y": PSUM pool for the y transposes
    ksq_src="sbuf",      # "psum" | "sbuf": k^2 input
    chunk_order="asc",   # "asc" | "desc": attention q-chunk processing order
    out_eng="scalar",    # "scalar" | "sync": queue for output stores
    osb_bufs=5,
    sched="serial",
    qk_tr="dma",
    y_tr="pe",
    tr_dma="sync",
    oproj_lag=3,
)


def build_nc(c_dim, t_dim, b_dim, **opts):
    """Build the per-core Bass module. All shapes in tokens/channels."""
    import concourse.mybir as mybir
    import concourse.tile as tile
    from concourse import bacc
    from concourse.masks import make_identity, make_upper_triangular

    o = dict(DEFAULT_OPTS)
    o.update(opts)
    QCH = o["att_chunk"]
    jpc = QCH // P  # j-blocks per attention chunk

    dt = mybir.dt
    f32 = dt.float32
    f16 = dt.float16
    bf16 = dt.bfloat16
    fp8 = dt.float8e4
    AF = mybir.ActivationFunctionType
    OP = mybir.AluOpType
    DR = mybir.MatmulPerfMode.DoubleRow

    n_ctiles = c_dim // P            # contraction tiles over C
    npairs = n_ctiles // 2
    ntok = b_dim * t_dim             # total token rows
    n_blocks_b = t_dim // P          # 128-tok blocks per batch
    n_qchunks = t_dim // QCH         # attention q chunks per batch
    blocks_per_chunk = TOK_CHUNK // P
    inv_sqrt_half = 1.0 / math.sqrt(HALF)
    VP = 272                         # V tile pitch (256 vals + 1 ones + pad)

    nc = bacc.Bacc()
    # x planes: [lo, hi]; w planes: [hi, lo] — cross-term DoubleRow pairs
    # (x_lo*w_hi + x_hi*w_lo) then use natural ascending slices on both.
    xt2 = nc.declare_dram_parameter("xt2", [2, c_dim, ntok], fp8, isOutput=False)
    # ramp tensors: host-prepacked partition-major (fully contiguous per
    # partition row) so the DMA model sees >=512B descriptors at full speed.
    # xm0..3: chunk-0 x in 128-tok pieces; wk2p: k cols both planes; wqv
    # plane-split.
    # rampk packs [x tok 0:128 | k cols] per (plane, ctile) so one DMA feeds
    # the first K-projection matmuls.
    rampk = nc.declare_dram_parameter("rampk", [P, 2 * n_ctiles * 384], fp8,
                                      isOutput=False)
    xms = {t: nc.declare_dram_parameter(f"xm{t}", [P, 2 * n_ctiles * P], fp8,
                                        isOutput=False)
           for t in range(1, blocks_per_chunk)}
    wqvh = nc.declare_dram_parameter("wqvh", [P, n_ctiles * 512], fp8,
                                     isOutput=False)
    wqvl = nc.declare_dram_parameter("wqvl", [P, n_ctiles * 512], fp8,
                                     isOutput=False)
    wot = nc.declare_dram_parameter("wot", [HEAD_DIM, c_dim], bf16, isOutput=False)
    lamneg = nc.declare_dram_parameter("lamneg", [P, 1], f32, isOutput=False)
    out = nc.declare_dram_parameter("out", [ntok, c_dim], f16, isOutput=True)

    xt_r = xt2.ap().rearrange("v (i p) t -> p v i t", p=P)    # [128,2,nct,ntok]
    wot_r = wot.ap().rearrange("(e p) n -> p e n", p=P)       # [128, 2, c_dim]

    with tile.TileContext(nc) as tc:
        with ExitStack() as ctx:
            # ---- persistent SBUF ----
            const_pool = ctx.enter_context(tc.tile_pool(name="const", bufs=1))
            # rampk_sb dim3: [x tok 0:128 | k cols 128:384]
            rampk_sb = const_pool.tile([P, 2, n_ctiles, 384], fp8,
                                       name="rampk_sb")
            wqv_sb = const_pool.tile([P, 2, n_ctiles, 512], fp8, name="wqv_sb")
            wot_sb = const_pool.tile([P, 2, c_dim], bf16, name="wot_sb")
            lam_sb = const_pool.tile([P, 1], f32, name="lam_sb")
            ident = const_pool.tile([P, P], bf16, name="ident")
            trimask = const_pool.tile([P, P], bf16, name="trimask")
            ones_sb = const_pool.tile([P, 1], bf16, name="ones_sb")
            nc.vector.memset(ones_sb[:], 1.0)

            xc_pool = ctx.enter_context(tc.tile_pool(name="xc", bufs=o["xc_bufs"]))

            # ---- ramp: chunk 0 comes in as part-major 128-tok minis
            # interleaved with the weight loads, so PE starts ~7us in and is
            # fed continuously. All transfers have >=512B descriptors.
            xm_sb = {t: const_pool.tile([P, 2, n_ctiles, P], fp8, name=f"xm{t}")
                     for t in range(1, blocks_per_chunk)}
            nc.sync.dma_start(rampk_sb[:],
                              rampk.ap().rearrange("p (v i n) -> p v i n",
                                                   v=2, i=n_ctiles))
            nc.sync.dma_start(wqv_sb[:, 0],
                              wqvh.ap().rearrange("p (i n) -> p i n",
                                                  i=n_ctiles))
            nc.sync.dma_start(xm_sb[1][:],
                              xms[1].ap().rearrange("p (v i n) -> p v i n",
                                                    v=2, i=n_ctiles))
            nc.sync.dma_start(wqv_sb[:, 1],
                              wqvl.ap().rearrange("p (i n) -> p i n",
                                                  i=n_ctiles))
            nc.sync.dma_start(xm_sb[2][:],
                              xms[2].ap().rearrange("p (v i n) -> p v i n",
                                                    v=2, i=n_ctiles))
            nc.sync.dma_start(xm_sb[3][:],
                              xms[3].ap().rearrange("p (v i n) -> p v i n",
                                                    v=2, i=n_ctiles))
            nc.scalar.dma_start(wot_sb[:], wot_r[:])
            nc.scalar.dma_start(lam_sb[:], lamneg.ap())
            make_identity(nc, ident[:])
            # 1.0 where kk <= q (partition <= free), else 0
            make_upper_triangular(nc, trimask[:], val=1.0, diag=True)

            qt_pool = ctx.enter_context(tc.tile_pool(name="qt", bufs=2))
            kt_pool = ctx.enter_context(tc.tile_pool(name="kt", bufs=2))
            ksq_pool = ctx.enter_context(tc.tile_pool(name="ksq", bufs=2))
            kscale_pool = ctx.enter_context(tc.tile_pool(name="kscale", bufs=2))
            v_pool = ctx.enter_context(tc.tile_pool(name="v", bufs=2))
            yt_pool = ctx.enter_context(tc.tile_pool(name="yt", bufs=2))
            pt_pool = ctx.enter_context(tc.tile_pool(name="pt", bufs=o["pt_bufs"]))
            y0_pool = ctx.enter_context(tc.tile_pool(name="y0", bufs=o["y0_mult"] * jpc))
            osb_pool = ctx.enter_context(tc.tile_pool(name="osb", bufs=o.get("osb_bufs", 3)))
            qn_pool = ctx.enter_context(tc.tile_pool(name="qn", bufs=o["qn_bufs"]))
            sq_pool = ctx.enter_context(tc.tile_pool(name="sq", bufs=2))
            rms_pool = ctx.enter_context(tc.tile_pool(name="rms", bufs=o["rms_bufs"]))
            nproj, nst, ny = o["psum"]
            psum_proj = ctx.enter_context(
                tc.tile_pool(name="psum_proj", bufs=nproj, space="PSUM"))
            psum_st = ctx.enter_context(
                tc.tile_pool(name="psum_st", bufs=nst, space="PSUM"))
            psum_y = ctx.enter_context(
                tc.tile_pool(name="psum_y", bufs=ny, space="PSUM"))

            tr_psum = psum_st if o["tr_pool"] == "st" else psum_proj
            tr_tag = o["tr_pool"] if o["tr_pool"] == "st" else "pp"
            tr_shape = QCH if o["tr_pool"] == "st" else 512

            def pe_transpose(dst_ap, src_ap):
                trp = tr_psum.tile([P, tr_shape], bf16, tag=tr_tag,
                                   name="trp")[:, :P]
                nc.tensor.transpose(trp, src_ap, ident[:])
                nc.vector.tensor_copy(dst_ap, trp)

            def pe_transpose_y(dst_ap, src_ap):
                trp = psum_y.tile([P, 257], bf16, tag="y", name="trpy")[:, :P]
                nc.tensor.transpose(trp, src_ap, ident[:])
                nc.vector.tensor_copy(dst_ap, trp)

            def dma_transpose(dst_ap, src_ap):
                tr_dma_eng = nc.sync if o["tr_dma"] == "sync" else nc.scalar
                tr_dma_eng.dma_start_transpose(out=dst_ap, in_=src_ap)

            tr_qk = pe_transpose if o["qk_tr"] == "pe" else dma_transpose
            tr_y = ((pe_transpose_y if o["ytr_pool"] == "y" else pe_transpose)
                    if o["y_tr"] == "pe" else dma_transpose)

            def dr_proj(out_ap, x_ap, w_ap, x_stationary):
                """3-term residual fp8 accumulation into out_ap [128, N].

                x_ap(pl, i, sl): plane/ctile/slice accessor; same for w_ap.
                Planes: x [lo, hi], w [hi, lo]. Emits 1.5*npairs DoubleRow
                matmuls; caller's region gets start on the first, stop on the
                last.
                """
                calls = []
                for pr in range(npairs):  # hi @ hi, k-tile pairs
                    calls.append((x_ap(1, slice(2 * pr, 2 * pr + 2)),
                                  w_ap(0, slice(2 * pr, 2 * pr + 2))))
                for i in range(n_ctiles):  # x_lo@w_hi + x_hi@w_lo per tile
                    calls.append((x_ap(slice(0, 2), i),
                                  w_ap(slice(0, 2), i)))
                n = len(calls)
                for idx, (xs, ws) in enumerate(calls):
                    lhsT, rhs = (xs, ws) if x_stationary else (ws, xs)
                    nc.tensor.matmul(out_ap, lhsT, rhs,
                                     start=(idx == 0), stop=(idx == n - 1),
                                     perf_mode=DR)

            def rsqrt_newton(dst, m, tmp_pool, width):
                """DVE-only rsqrt on the MSCALE-shifted mean-square range.
                Returns the tile holding the result (may be a fresh tile)."""
                t1 = tmp_pool.tile([P, width], f32, tag="rms", name="rsq_t1")
                nc.vector.tensor_tensor(t1[:], m, m, op=OP.mult)
                nc.vector.tensor_scalar(dst, m, RSQ_B, RSQ_A, OP.mult, OP.add)
                nc.vector.scalar_tensor_tensor(dst, t1[:], RSQ_C, dst,
                                               op0=OP.mult, op1=OP.add)
                nc.vector.tensor_scalar_max(dst, dst, RSQ_CLAMP)
                for _ in range(2):
                    nc.vector.tensor_tensor(t1[:], dst, dst, op=OP.mult)
                    nc.vector.scalar_tensor_tensor(t1[:], t1[:], -0.5, m,
                                                   op0=OP.mult, op1=OP.mult)
                    nc.vector.tensor_scalar(t1[:], t1[:], 1.0, 1.5,
                                            OP.mult, OP.add)
                    nc.vector.tensor_tensor(dst, dst, t1[:], op=OP.mult)

            for b in range(b_dim):
                qt_sb = qt_pool.tile([P, 2, t_dim], bf16, name=f"qt_b{b}", tag="qt")
                kt_sb = kt_pool.tile([P, 2, t_dim], bf16, name=f"kt_b{b}", tag="kt")
                v_sb = v_pool.tile([P, n_blocks_b, VP], bf16, name=f"v_b{b}", tag="v")
                yt_sb = yt_pool.tile([P, 2, t_dim], bf16, name=f"yt_b{b}", tag="yt")
                kscale_sb = kscale_pool.tile([P, 2 * n_blocks_b], f32,
                                             name=f"ksc_b{b}", tag="ksc")

                # ================= projections =================
                def do_proj_chunk(ch):
                    tok0 = b * t_dim + ch * TOK_CHUNK
                    first_ch = (b == 0 and ch == 0)
                    if first_ch:
                        xc = None
                    else:
                        xc = xc_pool.tile([P, 2, n_ctiles, TOK_CHUNK], fp8,
                                          tag="xc")
                        nc.sync.dma_start(xc[:], xt_r[:, :, :, tok0:tok0 + TOK_CHUNK])

                    # --- K projection straight into [d, tok] layout ---
                    # chunk 0 in 128-tok pieces (one per ramp mini)
                    kpieces = ([(0, P, rampk_sb, 0)]
                               + [(t * P, P, xm_sb[t], 0)
                                  for t in range(1, blocks_per_chunk)]
                               if first_ch
                               else [(h2 * 256, 256, xc, h2 * 256)
                                     for h2 in range(TOK_CHUNK // 256)])
                    kssq = psum_st.tile([P, QCH], f32, tag="st",
                                        name="kssq")[:, :8]
                    for v in range(2):
                        ktp = psum_proj.tile([P, 512], f32, tag="pp", name="ktp")
                        for off, wdt, src_t, soff in kpieces:
                            dr_proj(
                                ktp[:, off:off + wdt],
                                lambda pl, i, src_t=src_t, soff=soff, wdt=wdt:
                                    src_t[:, pl, i, soff:soff + wdt],
                                lambda pl, i, v=v: rampk_sb[
                                    :, pl, i, P + v * P:P + (v + 1) * P],
                                x_stationary=False)
                        ktdst = kt_sb[:, v, ch * TOK_CHUNK:(ch + 1) * TOK_CHUNK]
                        if o["ktcopy_eng"] == "act":
                            nc.scalar.copy(ktdst, ktp[:])
                        else:
                            nc.vector.tensor_copy(ktdst, ktp[:])
                        ksq = ksq_pool.tile([P, TOK_CHUNK], bf16, tag="ksq")
                        ksrc = ktdst if o["ksq_src"] == "sbuf" else ktp[:]
                        if o["ksq_eng"] == "act":
                            nc.scalar.activation(ksq[:], ksrc, AF.Square)
                        elif o["ksq_eng"] == "pool":
                            nc.gpsimd.tensor_tensor(ksq[:], ksrc, ksrc,
                                                    op=OP.mult)
                        else:
                            nc.vector.tensor_tensor(ksq[:], ksrc, ksrc,
                                                    op=OP.mult)
                        for t in range(blocks_per_chunk):
                            nc.tensor.matmul(
                                kssq[:, 2 * t + v:2 * t + v + 1],
                                ksq[:, t * P:(t + 1) * P], ones_sb[:],
                                start=True, stop=True)
                    # kscale = (1/64)/sqrt(mean(k^2) + eps) * inv_sqrt_half
                    ksl = kscale_sb[:, ch * 2 * blocks_per_chunk:
                                    (ch + 1) * 2 * blocks_per_chunk]
                    km = rms_pool.tile([P, 8], f32, tag="rms", name="km")
                    nc.vector.tensor_scalar(km[:], kssq[:], 1.0 / HALF,
                                            MSCALE * RMS_EPS, OP.mult, OP.add)
                    rsqrt_newton(ksl, km[:], rms_pool, 8)
                    nc.vector.tensor_scalar_mul(ksl, ksl, inv_sqrt_half)

                    for tl in range(blocks_per_chunk):
                        tb = ch * blocks_per_chunk + tl
                        # one bank: [q1 q2 | v]
                        qv = psum_proj.tile([P, 512], f32, tag="pp", name="qv")
                        xsrc = ((rampk_sb if tl == 0 else xm_sb[tl])
                                if first_ch else xc)
                        tsl = (slice(0, P) if first_ch
                               else slice(tl * P, (tl + 1) * P))
                        for h2 in range(2):
                            dr_proj(
                                qv[:, h2 * 256:(h2 + 1) * 256],
                                lambda pl, i, xsrc=xsrc, tsl=tsl:
                                    xsrc[:, pl, i, tsl],
                                lambda pl, i, h2=h2: wqv_sb[
                                    :, pl, i, h2 * 256:(h2 + 1) * 256],
                                x_stationary=True)
                        halves = [qv[:, 0:128], qv[:, 128:256]]
                        rmsg = rms_pool.tile([P, 2], f32, tag="rms")
                        for j, h in enumerate(halves):
                            sq = sq_pool.tile([P, P], bf16, tag="sq", name="sq")
                            nc.scalar.activation(
                                sq[:], h, AF.Square,
                                accum_out=rmsg[:, j:j + 1])
                        nc.vector.tensor_scalar(rmsg[:], rmsg[:], 1.0 / HALF,
                                                MSCALE * RMS_EPS, OP.mult, OP.add)
                        yv = rms_pool.tile([P, 2], f32, tag="rms")
                        rsqrt_newton(yv[:], rmsg[:], rms_pool, 2)
                        for j, h in enumerate(halves):
                            qn = qn_pool.tile([P, P], bf16, tag="qn")
                            nc.vector.tensor_scalar_mul(qn[:], h, yv[:, j:j + 1])
                            tr_qk(qt_sb[:, j, tb * P:(tb + 1) * P], qn[:])
                        # V (+ ones column for the softmax denominator)
                        vsrc = qv[:, 256:512]
                        if o["vcopy"] == "act":
                            nc.scalar.copy(v_sb[:, tb, 0:256], vsrc)
                        else:
                            nc.vector.tensor_copy(v_sb[:, tb, 0:256], vsrc)
                        nc.vector.memset(v_sb[:, tb, 256:257], 1.0)

                # ================= attention =================
                def do_att_chunk(cqi):
                    q0 = cqi * QCH
                    jmax = jpc * cqi + (jpc - 1)   # top kk-tile in this chunk
                    y0s = []
                    for v in range(2):
                        ys = [psum_y.tile([P, 257], f32, tag="y", name="ys")
                              for _ in range(jpc)]
                        for i in range(jmax + 1):
                            # jj0: first valid j-slot for this row (causal)
                            jj0 = max(0, i - jpc * cqi) if o["narrow_top"] else 0
                            w = QCH - jj0 * P
                            st = psum_st.tile([P, QCH], f32, tag="st",
                                              name="st")[:, :w]
                            nc.tensor.matmul(
                                st[:], kt_sb[:, v, i * P:(i + 1) * P],
                                qt_sb[:, v, q0 + jj0 * P:q0 + QCH],
                                start=True, stop=True)
                            pt = pt_pool.tile([P, QCH], bf16, tag="pt", name="pt")[:, :w]
                            nc.scalar.activation(
                                pt[:], st[:], AF.Exp,
                                scale=kscale_sb[:, 2 * i + v:2 * i + v + 1])
                            dj = i - jpc * cqi    # diagonal j-slot if >= 0
                            if dj >= 0:
                                tri_eng = (nc.gpsimd if o["trimask_eng"] == "pool"
                                           else nc.vector)
                                tri_eng.tensor_tensor(
                                    pt[:, (dj - jj0) * P:(dj - jj0 + 1) * P],
                                    pt[:, (dj - jj0) * P:(dj - jj0 + 1) * P],
                                    trimask[:], op=OP.mult)
                            for jj in range(jj0, jpc):
                                j = jpc * cqi + jj
                                if i > j:
                                    continue
                                nc.tensor.matmul(
                                    ys[jj][:],
                                    pt[:, (jj - jj0) * P:(jj - jj0 + 1) * P],
                                    v_sb[:, i, 0:257],
                                    start=(i == 0), stop=(i == j))
                        # epilogue for this view
                        for jj in range(jpc):
                            j = jpc * cqi + jj
                            inv = rms_pool.tile([P, 1], f32, tag="inv")
                            nc.vector.reciprocal(inv[:], ys[jj][:, 256:257])
                            if v == 0:
                                y0 = y0_pool.tile([P, 256], f32, tag="y0")
                                nc.vector.tensor_scalar_mul(
                                    y0[:], ys[jj][:, 0:256], inv[:])
                                y0s.append(y0)
                            else:
                                sc2 = rms_pool.tile([P, 1], f32, tag="inv")
                                nc.vector.tensor_tensor(
                                    sc2[:], inv[:], lam_sb[:], op=OP.mult)
                                yf = qn_pool.tile([P, 256], bf16, tag="yf")
                                nc.vector.scalar_tensor_tensor(
                                    yf[:], ys[jj][:, 0:256], sc2[:], y0s[jj][:],
                                    op0=OP.mult, op1=OP.add)
                                for e in range(2):
                                    tr_y(yt_sb[:, e, j * P:(j + 1) * P],
                                         yf[:, e * P:(e + 1) * P])

                    # ---- output projection, lagged one chunk ----
                    # Fills PE's exp-wait gaps with oproj matmuls; the lag
                    # keeps oproj off the tail of its own yt transpose chain.
                    out_eng = nc.scalar if o["out_eng"] == "scalar" else nc.sync
                    lag = o["oproj_lag"]
                    if cqi < lag:
                        obs = []
                    elif cqi == n_qchunks - 1:
                        obs = list(range(jpc * (cqi - lag), jpc * (cqi + 1)))
                    else:
                        obs = list(range(jpc * (cqi - lag), jpc * (cqi - lag + 1)))
                    for tb in obs:
                        row0 = b * t_dim + tb * P
                        merged = o["osb_merge"]
                        orow = (osb_pool.tile([P, c_dim], f16, tag="orow",
                                              name="orow")
                                if merged else None)
                        for cc in range(c_dim // 512):
                            op_ps = psum_proj.tile([P, 512], f32, tag="pp",
                                                   name="ops")
                            for e in range(2):
                                nc.tensor.matmul(
                                    op_ps[:], yt_sb[:, e, tb * P:(tb + 1) * P],
                                    wot_sb[:, e, cc * 512:(cc + 1) * 512],
                                    start=(e == 0), stop=(e == 1))
                            osb = (orow[:, cc * 512:(cc + 1) * 512] if merged
                                   else osb_pool.tile([P, 512], f16, tag="osb"))
                            oc = o["oproj_copy"]
                            if oc == "alt":
                                oc = "act" if (tb + cc) % 2 == 0 else "dve"

                            elif oc == "rotdp":
                                oc = ("dve", "act")[(tb * 4 + cc) % 2]
                            if oc == "act":
                                nc.scalar.copy(osb, op_ps[:])
                            else:
                                nc.vector.tensor_copy(osb, op_ps[:])
                            if not merged:
                                out_eng.dma_start(
                                    out.ap()[row0:row0 + P,
                                             cc * 512:(cc + 1) * 512], osb)
                        if merged:
                            out_eng.dma_start(
                                out.ap()[row0:row0 + P, :], orow[:])

                # ---- schedule: interleave attention chunks between proj
                # chunks (deps allow A_c once proj chunk (2c+1)//(2*jpc) is
                # done; hold one extra proj chunk of slack) ----
                n_pchunks = t_dim // TOK_CHUNK
                if o["sched"] == "interleave":
                    tokens = []
                    nexta = 0
                    for ch in range(n_pchunks):
                        tokens.append(("P", ch))
                        while (nexta < n_qchunks
                               and (jpc * nexta + jpc - 1) // blocks_per_chunk
                               <= ch - 1):
                            tokens.append(("A", nexta))
                            nexta += 1
                    tokens += [("A", c) for c in range(nexta, n_qchunks)]
                else:
                    tokens = ([("P", ch) for ch in range(n_pchunks)]
                              + [("A", c) for c in range(n_qchunks)])
                for kind, idx in tokens:
                    (do_proj_chunk if kind == "P" else do_att_chunk)(idx)
    nc.compile()
    return nc


_NC_CACHE = {}
TRACE = False        # set True (e.g. from test.py) to capture an NTFF profile
LAST_RESULT = None   # BassKernelResults of the most recent run


def _get_nc(c_dim, t_dim, b_dim):
    key = (c_dim, t_dim, b_dim)
    if key not in _NC_CACHE:
        _NC_CACHE[key] = build_nc(c_dim, t_dim, b_dim)
    return _NC_CACHE[key]


def prep_inputs(x, wq, wk, wv, wo, lq1, lk1, lq2, lk2):
    """Host-side prep: per-core input maps."""
    import ml_dtypes

    bf16 = ml_dtypes.bfloat16
    fp8 = ml_dtypes.float8_e4m3
    b_dim, t_dim, c_dim = x.shape

    lam1 = np.exp(np.sum(lq1.astype(np.float64) * lk1.astype(np.float64)))
    lam2 = np.exp(np.sum(lq2.astype(np.float64) * lk2.astype(np.float64)))
    lam_full = np.float32(lam1 - lam2 + LAMBDA_INIT)

    n_ctiles = c_dim // P
    xt = np.ascontiguousarray(x.reshape(b_dim * t_dim, c_dim).T)
    xh = xt.astype(fp8)
    xl = (xt - xh.astype(np.float32)).astype(fp8)
    xt2 = np.ascontiguousarray(np.stack([xl, xh]))  # planes [lo, hi]
    lamneg = np.full((P, 1), -lam_full, dtype=np.float32)

    def part_major(a):
        """[..., 2?, C, n] -> [128, prod(rest)] with c = i*128 + p."""
        if a.ndim == 2:
            a = a[None]
        v, cdim, n = a.shape
        a = a.reshape(v, n_ctiles, P, n).transpose(2, 0, 1, 3)
        return np.ascontiguousarray(a.reshape(P, v * n_ctiles * n))

    xm = {f"xm{t}": part_major(xt2[:, :, t * P:(t + 1) * P])
          for t in range(1, TOK_CHUNK // P)}

    in_maps = []
    for h in range(N_CORES):
        sl = slice(h * HEAD_DIM, (h + 1) * HEAD_DIM)
        wk64 = wk[sl].T * np.float32(WSCALE)
        wkh = wk64.astype(fp8)
        wkl = (wk64 - wkh.astype(np.float32)).astype(fp8)
        # rampk: [x tok 0:128 | k cols] per (plane, ctile)
        rampk = part_major(np.concatenate(
            [xt2[:, :, 0:P].astype(fp8), np.stack([wkh, wkl])], axis=2))
        wqv64 = np.concatenate([wq[sl].T, wv[sl].T], axis=1) * np.float32(WSCALE)
        wqvh_ = wqv64.astype(fp8)
        wqvl_ = (wqv64 - wqvh_.astype(np.float32)).astype(fp8)
        wot_h = np.ascontiguousarray(
            (wo[:, sl] * ((1.0 - LAMBDA_INIT) / WSCALE)).T).astype(bf16)
        in_maps.append({
            "xt2": xt2, "rampk": rampk,
            "wqvh": part_major(wqvh_), "wqvl": part_major(wqvl_),
            "wot": wot_h, "lamneg": lamneg, **xm,
        })
    return in_maps


_FN_CACHE = {}


def _get_callable(nc):
    """Build (once) a reusable jitted shard_map callable for this module —
    mirrors bass2jax.run_bass_via_pjrt's multi-core path, but cached so
    repeat kernel() calls skip retracing."""
    if id(nc) in _FN_CACHE:
        return _FN_CACHE[id(nc)]
    import jax
    from jax.sharding import Mesh, PartitionSpec, NamedSharding
    from jax.experimental.shard_map import shard_map
    import concourse.mybir as mybir
    import concourse.bass2jax as b2j

    b2j.install_neuronx_cc_hook()
    pname = nc.partition_id_tensor.name if nc.partition_id_tensor else None
    in_names, out_names, out_avals, zero_shapes = [], [], [], []
    for alloc in nc.m.functions[0].allocations:
        if not isinstance(alloc, mybir.MemoryLocationSet):
            continue
        name = alloc.memorylocations[0].name
        if alloc.kind == "ExternalInput":
            if name != pname:
                in_names.append(name)
        elif alloc.kind == "ExternalOutput":
            out_names.append(name)
            shape = tuple(alloc.tensor_shape)
            dtype = mybir.dt.np(alloc.dtype)
            out_avals.append(jax.core.ShapedArray(shape, dtype))
            zero_shapes.append((shape, dtype))
    n_params = len(in_names)
    all_in = in_names + out_names
    if pname is not None:
        all_in = all_in + [pname]

    def _body(*args):
        operands = list(args)
        if pname is not None:
            operands.append(b2j.partition_id_tensor())
        return tuple(b2j._bass_exec_p.bind(
            *operands,
            out_avals=tuple(out_avals),
            in_names=tuple(all_in),
            out_names=tuple(out_names),
            lowering_input_output_aliases=(),
            sim_require_finite=True,
            sim_require_nnan=True,
            nc=nc,
        ))

    devices = jax.devices()[:N_CORES]
    mesh = Mesh(np.asarray(devices), ("core",))
    nio = n_params + len(out_names)
    fn = jax.jit(shard_map(_body, mesh=mesh,
                           in_specs=(PartitionSpec("core"),) * nio,
                           out_specs=(PartitionSpec("core"),) * len(out_names),
                           check_rep=False),
                 donate_argnums=tuple(range(n_params, nio)), keep_unused=True)
    sh = NamedSharding(mesh, PartitionSpec("core"))
    entry = (fn, in_names, out_names, zero_shapes, sh)
    _FN_CACHE[id(nc)] = entry
    return entry


def kernel(x, wq, wk, wv, wo, lq1, lk1, lq2, lk2):
    b_dim, t_dim, c_dim = x.shape
    in_maps = prep_inputs(x, wq, wk, wv, wo, lq1, lk1, lq2, lk2)
    nc = _get_nc(c_dim, t_dim, b_dim)

    try:
        import jax
        fn, in_names, out_names, zero_shapes, sh = _get_callable(nc)
        concat_in = [
            np.concatenate([np.asarray(in_maps[c][n]) for c in range(N_CORES)],
                           axis=0) for n in in_names]
        concat_zeros = [np.zeros((N_CORES * s[0], *s[1:]), d)
                        for s, d in zero_shapes]
        dev_in = [jax.device_put(a, sh) for a in concat_in]
        dev_zero = [jax.device_put(a, sh) for a in concat_zeros]
        outs = fn(*dev_in, *dev_zero)
        arr = np.asarray(outs[out_names.index("out")])
        acc = arr.reshape(N_CORES, b_dim * t_dim, c_dim).astype(
            np.float32).sum(axis=0)
    except Exception:
        from concourse.bass_utils import run_bass_kernel_spmd
        res = run_bass_kernel_spmd(nc, in_maps, list(range(N_CORES)),
                                   trace=TRACE)
        global LAST_RESULT
        LAST_RESULT = res
        acc = np.zeros((b_dim * t_dim, c_dim), dtype=np.float32)
        for h in range(N_CORES):
            acc += res.results[h]["out"].astype(np.float32)
    return acc.reshape(b_dim, t_dim, c_dim)


# revision 40
# speedup vs baseline: 1.1991x; 1.0059x over previous
"""Differential attention (B=2, T=2048, C=2048, 8 heads x 256) on 8 trn2 cores.

Sharding: tensor-parallel over the 8 effective heads — core h computes head h's
projections + attention and a partial output projection; host sums partials.

Projections run in fp8e4m3 with a 3-term residual (hi/lo planes of both x and
w, DoubleRow matmuls): (xh+xl)@(wh+wl) ~ xh@wh + (xh@wl + xl@wh), each pair of
128-contraction tiles fused into one DoubleRow instruction at 0.5 cycles/row —
25% fewer PE cycles than bf16 at bf16-level accuracy. Weights are host-scaled
by 64 into fp8's normal range; q/k absorb the scale in rms-norm (the Newton
rsqrt seed is refit for the 4096x mean-square), v stays 64x and the out-proj
weights carry 1/64.

Attention (scores S.T = K.T@Q -> exp -> P.T @ V with a ones-column giving the
softmax denominator) stays bf16: P = exp(s) spans e^-inf..e^11 which fp8
cannot represent, and bf16 q/k are needed for exp accuracy. Causal blocks
skipped; diagonal blocks masked multiplicatively post-exp.

Output is stored fp16 (halves store DMA); host sums the 8 partials in f32.
"""

import math
from contextlib import ExitStack

import numpy as np

# ---- problem constants (hardcoded per the harness contract) ----
B = 2
T = 2048
C = 2048
N_HEAD = 8
HEAD_DIM = 256
HALF = 128
LAMBDA_INIT = 0.8
RMS_EPS = 1.1920929e-07
N_CORES = 8

P = 128          # partitions
TOK_CHUNK = 512  # projection tok chunk (DMA granularity)
WSCALE = 64.0    # host-side weight scale into fp8 normal range
MSCALE = WSCALE * WSCALE  # mean-square scale (4096)

# Newton rsqrt seed: least-squares quadratic fit of rsqrt on m in
# MSCALE*[0.3, 2.0] (q/k mean-squares after the 64x weight scale), clamped.
RSQ_A = 2.07556761 / WSCALE
RSQ_B = -1.47991565 / (MSCALE * WSCALE)
RSQ_C = 0.41306651 / (MSCALE * MSCALE * WSCALE)
RSQ_CLAMP = 0.05 / WSCALE

DEFAULT_OPTS = dict(
    att_chunk=256,       # attention q-chunk width (256 or 512)
    oproj_copy="dve",    # out-proj PSUM->SBUF evac: act|dve|alt (PSUM: no pool)
    trimask_eng="dve",   # "dve" | "pool": diagonal-block mask multiply
    psum=(3, 3, 2),      # banks: (proj, st, y) — must sum to <= 8
    pt_bufs=12,          # P.T tile double-buffer depth
    xc_bufs=2,           # x chunk prefetch depth
    vcopy="dve",         # "act" | "dve": V PSUM->SBUF copy engine (PSUM: no pool)
    osb_merge=True,      # one output-store DMA per tok block (vs per c-chunk)
    narrow_top=True,     # compute only the valid half of the top causal row
    tr_pool="st",        # "st" | "pp": PSUM pool used by PE transposes
    rms_bufs=4,
    qn_bufs=6,
    y0_mult=2,
    ksq_eng="dve",       # "act" | "dve": engine computing k^2
    ktcopy_eng="act",    # "act" | "dve": engine evacuating KT psum
    ytr_pool="y",        # "st" | "y": PSUM pool for the y transposes
    ksq_src="sbuf",      # "psum" | "sbuf": k^2 input
    chunk_order="asc",   # "asc" | "desc": attention q-chunk processing order
    out_eng="scalar",    # "scalar" | "sync": queue for output stores
    osb_bufs=5,
    sched="serial",
    qk_tr="dma",
    y_tr="pe",
    tr_dma="sync",
    tail_alt=False,
    oproj_lag=3,
)


def build_nc(c_dim, t_dim, b_dim, **opts):
    """Build the per-core Bass module. All shapes in tokens/channels."""
    import concourse.mybir as mybir
    import concourse.tile as tile
    from concourse import bacc
    from concourse.masks import make_identity, make_upper_triangular

    o = dict(DEFAULT_OPTS)
    o.update(opts)
    QCH = o["att_chunk"]
    jpc = QCH // P  # j-blocks per attention chunk

    dt = mybir.dt
    f32 = dt.float32
    f16 = dt.float16
    bf16 = dt.bfloat16
    fp8 = dt.float8e4
    AF = mybir.ActivationFunctionType
    OP = mybir.AluOpType
    DR = mybir.MatmulPerfMode.DoubleRow

    n_ctiles = c_dim // P            # contraction tiles over C
    npairs = n_ctiles // 2
    ntok = b_dim * t_dim             # total token rows
    n_blocks_b = t_dim // P          # 128-tok blocks per batch
    n_qchunks = t_dim // QCH         # attention q chunks per batch
    blocks_per_chunk = TOK_CHUNK // P
    inv_sqrt_half = 1.0 / math.sqrt(HALF)
    VP = 272                         # V tile pitch (256 vals + 1 ones + pad)

    nc = bacc.Bacc()
    # x and w planes are both [hi, lo]; cross-term DoubleRows pair over the
    # ctile dim so any single plane is usable as soon as its DMA lands.
    xt2 = nc.declare_dram_parameter("xt2", [2, c_dim, ntok], fp8, isOutput=False)
    # ramp tensors: host-prepacked partition-major (fully contiguous per
    # partition row) so the DMA model sees >=512B descriptors at full speed.
    # xm0..3: chunk-0 x in 128-tok pieces; wk2p: k cols both planes; wqv
    # plane-split.
    # rampk packs [x tok 0:128 | k cols] per (plane, ctile) so one DMA feeds
    # the first K-projection matmuls.
    rampk = nc.declare_dram_parameter("rampk", [P, 2 * n_ctiles * 384], fp8,
                                      isOutput=False)
    xms = {t: nc.declare_dram_parameter(f"xm{t}", [P, 2 * n_ctiles * P], fp8,
                                        isOutput=False)
           for t in range(1, blocks_per_chunk)}
    wqvh = nc.declare_dram_parameter("wqvh", [P, n_ctiles * 512], fp8,
                                     isOutput=False)
    wqvl = nc.declare_dram_parameter("wqvl", [P, n_ctiles * 512], fp8,
                                     isOutput=False)
    wot = nc.declare_dram_parameter("wot", [HEAD_DIM, c_dim], bf16, isOutput=False)
    lamneg = nc.declare_dram_parameter("lamneg", [P, 1], f32, isOutput=False)
    out = nc.declare_dram_parameter("out", [ntok, c_dim], f16, isOutput=True)

    xt_r = xt2.ap().rearrange("v (i p) t -> p v i t", p=P)    # [128,2,nct,ntok]
    wot_r = wot.ap().rearrange("(e p) n -> p e n", p=P)       # [128, 2, c_dim]

    with tile.TileContext(nc) as tc:
        with ExitStack() as ctx:
            # ---- persistent SBUF ----
            const_pool = ctx.enter_context(tc.tile_pool(name="const", bufs=1))
            # rampk_sb dim3: [x tok 0:128 | k cols 128:384]
            rampk_sb = const_pool.tile([P, 2, n_ctiles, 384], fp8,
                                       name="rampk_sb")
            wqv_sb = const_pool.tile([P, 2, n_ctiles, 512], fp8, name="wqv_sb")
            wot_sb = const_pool.tile([P, 2, c_dim], bf16, name="wot_sb")
            lam_sb = const_pool.tile([P, 1], f32, name="lam_sb")
            ident = const_pool.tile([P, P], bf16, name="ident")
            trimask = const_pool.tile([P, P], bf16, name="trimask")
            ones_sb = const_pool.tile([P, 1], bf16, name="ones_sb")
            nc.vector.memset(ones_sb[:], 1.0)

            xc_pool = ctx.enter_context(tc.tile_pool(name="xc", bufs=o["xc_bufs"]))

            # ---- ramp: chunk 0 comes in as part-major 128-tok minis
            # interleaved with the weight loads, so PE starts ~7us in and is
            # fed continuously. All transfers have >=512B descriptors.
            xm_sb = {t: const_pool.tile([P, 2, n_ctiles, P], fp8, name=f"xm{t}")
                     for t in range(1, blocks_per_chunk)}
            rampk_r = rampk.ap().rearrange("p (v i n) -> p v i n",
                                           v=2, i=n_ctiles)
            nc.sync.dma_start(rampk_sb[:, 0], rampk_r[:, 0])
            nc.sync.dma_start(rampk_sb[:, 1], rampk_r[:, 1])
            nc.sync.dma_start(wqv_sb[:, 0],
                              wqvh.ap().rearrange("p (i n) -> p i n",
                                                  i=n_ctiles))
            nc.sync.dma_start(wqv_sb[:, 1],
                              wqvl.ap().rearrange("p (i n) -> p i n",
                                                  i=n_ctiles))
            for t in range(1, blocks_per_chunk):
                xm_r = xms[t].ap().rearrange("p (v i n) -> p v i n",
                                             v=2, i=n_ctiles)
                nc.sync.dma_start(xm_sb[t][:, 0], xm_r[:, 0])
                nc.sync.dma_start(xm_sb[t][:, 1], xm_r[:, 1])
            nc.scalar.dma_start(wot_sb[:], wot_r[:])
            nc.scalar.dma_start(lam_sb[:], lamneg.ap())
            make_identity(nc, ident[:])
            # 1.0 where kk <= q (partition <= free), else 0
            make_upper_triangular(nc, trimask[:], val=1.0, diag=True)

            qt_pool = ctx.enter_context(tc.tile_pool(name="qt", bufs=2))
            kt_pool = ctx.enter_context(tc.tile_pool(name="kt", bufs=2))
            ksq_pool = ctx.enter_context(tc.tile_pool(name="ksq", bufs=2))
            kscale_pool = ctx.enter_context(tc.tile_pool(name="kscale", bufs=2))
            v_pool = ctx.enter_context(tc.tile_pool(name="v", bufs=2))
            yt_pool = ctx.enter_context(tc.tile_pool(name="yt", bufs=2))
            pt_pool = ctx.enter_context(tc.tile_pool(name="pt", bufs=o["pt_bufs"]))
            y0_pool = ctx.enter_context(tc.tile_pool(name="y0", bufs=o["y0_mult"] * jpc))
            osb_pool = ctx.enter_context(tc.tile_pool(name="osb", bufs=o.get("osb_bufs", 3)))
            qn_pool = ctx.enter_context(tc.tile_pool(name="qn", bufs=o["qn_bufs"]))
            sq_pool = ctx.enter_context(tc.tile_pool(name="sq", bufs=2))
            rms_pool = ctx.enter_context(tc.tile_pool(name="rms", bufs=o["rms_bufs"]))
            nproj, nst, ny = o["psum"]
            psum_proj = ctx.enter_context(
                tc.tile_pool(name="psum_proj", bufs=nproj, space="PSUM"))
            psum_st = ctx.enter_context(
                tc.tile_pool(name="psum_st", bufs=nst, space="PSUM"))
            psum_y = ctx.enter_context(
                tc.tile_pool(name="psum_y", bufs=ny, space="PSUM"))

            tr_psum = psum_st if o["tr_pool"] == "st" else psum_proj
            tr_tag = o["tr_pool"] if o["tr_pool"] == "st" else "pp"
            tr_shape = QCH if o["tr_pool"] == "st" else 512

            def pe_transpose(dst_ap, src_ap):
                trp = tr_psum.tile([P, tr_shape], bf16, tag=tr_tag,
                                   name="trp")[:, :P]
                nc.tensor.transpose(trp, src_ap, ident[:])
                nc.vector.tensor_copy(dst_ap, trp)

            def pe_transpose_y(dst_ap, src_ap):
                trp = psum_y.tile([P, 257], bf16, tag="y", name="trpy")[:, :P]
                nc.tensor.transpose(trp, src_ap, ident[:])
                nc.vector.tensor_copy(dst_ap, trp)

            def dma_transpose(dst_ap, src_ap):
                tr_dma_eng = nc.sync if o["tr_dma"] == "sync" else nc.scalar
                tr_dma_eng.dma_start_transpose(out=dst_ap, in_=src_ap)

            tr_qk = pe_transpose if o["qk_tr"] == "pe" else dma_transpose
            tr_y = ((pe_transpose_y if o["ytr_pool"] == "y" else pe_transpose)
                    if o["y_tr"] == "pe" else dma_transpose)

            def dr_proj(out_ap, x_ap, w_ap, x_stationary):
                """3-term residual fp8 accumulation into out_ap [128, N].

                x_ap(pl, i): plane/ctile accessor; same for w_ap. Both
                tensors are plane [hi, lo]; cross terms pair over the ctile
                dim so each call needs only one plane of each operand. Emits
                3*npairs DoubleRow matmuls; the caller's region gets start on
                the first, stop on the last.
                """
                calls = []
                for pr in range(npairs):  # hi @ hi, k-tile pairs
                    pair = slice(2 * pr, 2 * pr + 2)
                    calls.append((x_ap(0, pair), w_ap(0, pair)))
                for pr in range(npairs):  # x_lo @ w_hi pairs
                    pair = slice(2 * pr, 2 * pr + 2)
                    calls.append((x_ap(1, pair), w_ap(0, pair)))
                for pr in range(npairs):  # x_hi @ w_lo pairs
                    pair = slice(2 * pr, 2 * pr + 2)
                    calls.append((x_ap(0, pair), w_ap(1, pair)))
                n = len(calls)
                for idx, (xs, ws) in enumerate(calls):
                    lhsT, rhs = (xs, ws) if x_stationary else (ws, xs)
                    nc.tensor.matmul(out_ap, lhsT, rhs,
                                     start=(idx == 0), stop=(idx == n - 1),
                                     perf_mode=DR)

            def rsqrt_newton(dst, m, tmp_pool, width):
                """DVE-only rsqrt on the MSCALE-shifted mean-square range.
                Returns the tile holding the result (may be a fresh tile)."""
                t1 = tmp_pool.tile([P, width], f32, tag="rms", name="rsq_t1")
                nc.vector.tensor_tensor(t1[:], m, m, op=OP.mult)
                nc.vector.tensor_scalar(dst, m, RSQ_B, RSQ_A, OP.mult, OP.add)
                nc.vector.scalar_tensor_tensor(dst, t1[:], RSQ_C, dst,
                                               op0=OP.mult, op1=OP.add)
                nc.vector.tensor_scalar_max(dst, dst, RSQ_CLAMP)
                for _ in range(2):
                    nc.vector.tensor_tensor(t1[:], dst, dst, op=OP.mult)
                    nc.vector.scalar_tensor_tensor(t1[:], t1[:], -0.5, m,
                                                   op0=OP.mult, op1=OP.mult)
                    nc.vector.tensor_scalar(t1[:], t1[:], 1.0, 1.5,
                                            OP.mult, OP.add)
                    nc.vector.tensor_tensor(dst, dst, t1[:], op=OP.mult)

            for b in range(b_dim):
                qt_sb = qt_pool.tile([P, 2, t_dim], bf16, name=f"qt_b{b}", tag="qt")
                kt_sb = kt_pool.tile([P, 2, t_dim], bf16, name=f"kt_b{b}", tag="kt")
                v_sb = v_pool.tile([P, n_blocks_b, VP], bf16, name=f"v_b{b}", tag="v")
                yt_sb = yt_pool.tile([P, 2, t_dim], bf16, name=f"yt_b{b}", tag="yt")
                kscale_sb = kscale_pool.tile([P, 2 * n_blocks_b], f32,
                                             name=f"ksc_b{b}", tag="ksc")

                # ================= projections =================
                def do_proj_chunk(ch):
                    tok0 = b * t_dim + ch * TOK_CHUNK
                    first_ch = (b == 0 and ch == 0)
                    if first_ch:
                        xc = None
                    else:
                        xc = xc_pool.tile([P, 2, n_ctiles, TOK_CHUNK], fp8,
                                          tag="xc")
                        nc.sync.dma_start(xc[:], xt_r[:, :, :, tok0:tok0 + TOK_CHUNK])

                    # --- K projection straight into [d, tok] layout ---
                    # chunk 0 in 128-tok pieces (one per ramp mini)
                    kpieces = ([(0, P, rampk_sb, 0)]
                               + [(t * P, P, xm_sb[t], 0)
                                  for t in range(1, blocks_per_chunk)]
                               if first_ch
                               else [(h2 * 256, 256, xc, h2 * 256)
                                     for h2 in range(TOK_CHUNK // 256)])
                    kssq = psum_st.tile([P, QCH], f32, tag="st",
                                        name="kssq")[:, :8]
                    ktps = [psum_proj.tile([P, 512], f32, tag="pp", name="ktp")
                            for _ in range(2)]

                    def k_piece(v, piece):
                        off, wdt, src_t, soff = piece
                        dr_proj(
                            ktps[v][:, off:off + wdt],
                            lambda pl, i, src_t=src_t, soff=soff, wdt=wdt:
                                src_t[:, pl, i, soff:soff + wdt],
                            lambda pl, i, v=v: rampk_sb[
                                :, pl, i, P + v * P:P + (v + 1) * P],
                            x_stationary=False)

                    def k_evac():
                        for v in range(2):
                            ktp = ktps[v]
                            ktdst = kt_sb[:, v,
                                          ch * TOK_CHUNK:(ch + 1) * TOK_CHUNK]
                            if o["ktcopy_eng"] == "act":
                                nc.scalar.copy(ktdst, ktp[:])
                            else:
                                nc.vector.tensor_copy(ktdst, ktp[:])
                            ksq = ksq_pool.tile([P, TOK_CHUNK], bf16, tag="ksq")
                            ksrc = ktdst if o["ksq_src"] == "sbuf" else ktp[:]
                            if o["ksq_eng"] == "act":
                                nc.scalar.activation(ksq[:], ksrc, AF.Square)
                            elif o["ksq_eng"] == "pool":
                                nc.gpsimd.tensor_tensor(ksq[:], ksrc, ksrc,
                                                        op=OP.mult)
                            else:
                                nc.vector.tensor_tensor(ksq[:], ksrc, ksrc,
                                                        op=OP.mult)
                            for t in range(blocks_per_chunk):
                                nc.tensor.matmul(
                                    kssq[:, 2 * t + v:2 * t + v + 1],
                                    ksq[:, t * P:(t + 1) * P], ones_sb[:],
                                    start=True, stop=True)
                        # kscale = (1/64)/sqrt(mean(k^2)+eps) * inv_sqrt_half
                        ksl = kscale_sb[:, ch * 2 * blocks_per_chunk:
                                        (ch + 1) * 2 * blocks_per_chunk]
                        km = rms_pool.tile([P, 8], f32, tag="rms", name="km")
                        nc.vector.tensor_scalar(km[:], kssq[:], 1.0 / HALF,
                                                MSCALE * RMS_EPS,
                                                OP.mult, OP.add)
                        rsqrt_newton(ksl, km[:], rms_pool, 8)
                        nc.vector.tensor_scalar_mul(ksl, ksl, inv_sqrt_half)

                    if not first_ch:
                        for v in range(2):
                            for piece in kpieces:
                                k_piece(v, piece)
                        k_evac()

                    nc.vector.memset(
                        v_sb[:, ch * blocks_per_chunk:
                             (ch + 1) * blocks_per_chunk, 256:257], 1.0)
                    for tl in range(blocks_per_chunk):
                        tb = ch * blocks_per_chunk + tl
                        if first_ch:
                            # emit K piece tl just-in-time with its ramp mini
                            for v in range(2):
                                k_piece(v, kpieces[tl])
                        # one bank: [q1 q2 | v]
                        qv = psum_proj.tile([P, 512], f32, tag="pp", name="qv")
                        xsrc = ((rampk_sb if tl == 0 else xm_sb[tl])
                                if first_ch else xc)
                        tsl = (slice(0, P) if first_ch
                               else slice(tl * P, (tl + 1) * P))
                        for h2 in range(2):
                            dr_proj(
                                qv[:, h2 * 256:(h2 + 1) * 256],
                                lambda pl, i, xsrc=xsrc, tsl=tsl:
                                    xsrc[:, pl, i, tsl],
                                lambda pl, i, h2=h2: wqv_sb[
                                    :, pl, i, h2 * 256:(h2 + 1) * 256],
                                x_stationary=True)
                        halves = [qv[:, 0:128], qv[:, 128:256]]
                        rmsg = rms_pool.tile([P, 2], f32, tag="rms")
                        for j, h in enumerate(halves):
                            sq = sq_pool.tile([P, P], bf16, tag="sq", name="sq")
                            nc.scalar.activation(
                                sq[:], h, AF.Square,
                                accum_out=rmsg[:, j:j + 1])
                        nc.vector.tensor_scalar(rmsg[:], rmsg[:], 1.0 / HALF,
                                                MSCALE * RMS_EPS, OP.mult, OP.add)
                        yv = rms_pool.tile([P, 2], f32, tag="rms")
                        rsqrt_newton(yv[:], rmsg[:], rms_pool, 2)
                        for j, h in enumerate(halves):
                            qn = qn_pool.tile([P, P], bf16, tag="qn")
                            nc.vector.tensor_scalar_mul(qn[:], h, yv[:, j:j + 1])
                            tr_qk(qt_sb[:, j, tb * P:(tb + 1) * P], qn[:])
                        # V (+ ones column for the softmax denominator)
                        vsrc = qv[:, 256:512]
                        if o["vcopy"] == "act":
                            nc.scalar.copy(v_sb[:, tb, 0:256], vsrc)
                        else:
                            nc.vector.tensor_copy(v_sb[:, tb, 0:256], vsrc)
                    if first_ch:
                        k_evac()

                # ================= attention =================
                def do_att_chunk(cqi):
                    q0 = cqi * QCH
                    jmax = jpc * cqi + (jpc - 1)   # top kk-tile in this chunk
                    y0s = []
                    for v in range(2):
                        ys = [psum_y.tile([P, 257], f32, tag="y", name="ys")
                              for _ in range(jpc)]
                        for i in range(jmax + 1):
                            # jj0: first valid j-slot for this row (causal)
                            jj0 = max(0, i - jpc * cqi) if o["narrow_top"] else 0
                            w = QCH - jj0 * P
                            st = psum_st.tile([P, QCH], f32, tag="st",
                                              name="st")[:, :w]
                            nc.tensor.matmul(
                                st[:], kt_sb[:, v, i * P:(i + 1) * P],
                                qt_sb[:, v, q0 + jj0 * P:q0 + QCH],
                                start=True, stop=True)
                            pt = pt_pool.tile([P, QCH], bf16, tag="pt", name="pt")[:, :w]
                            nc.scalar.activation(
                                pt[:], st[:], AF.Exp,
                                scale=kscale_sb[:, 2 * i + v:2 * i + v + 1])
                            dj = i - jpc * cqi    # diagonal j-slot if >= 0
                            if dj >= 0:
                                tri_eng = (nc.gpsimd if o["trimask_eng"] == "pool"
                                           else nc.vector)
                                tri_eng.tensor_tensor(
                                    pt[:, (dj - jj0) * P:(dj - jj0 + 1) * P],
                                    pt[:, (dj - jj0) * P:(dj - jj0 + 1) * P],
                                    trimask[:], op=OP.mult)
                            for jj in range(jj0, jpc):
                                j = jpc * cqi + jj
                                if i > j:
                                    continue
                                nc.tensor.matmul(
                                    ys[jj][:],
                                    pt[:, (jj - jj0) * P:(jj - jj0 + 1) * P],
                                    v_sb[:, i, 0:257],
                                    start=(i == 0), stop=(i == j))
                        # epilogue for this view
                        for jj in range(jpc):
                            j = jpc * cqi + jj
                            inv = rms_pool.tile([P, 1], f32, tag="inv")
                            nc.vector.reciprocal(inv[:], ys[jj][:, 256:257])
                            if v == 0:
                                y0 = y0_pool.tile([P, 256], f32, tag="y0")
                                nc.vector.tensor_scalar_mul(
                                    y0[:], ys[jj][:, 0:256], inv[:])
                                y0s.append(y0)
                            else:
                                sc2 = rms_pool.tile([P, 1], f32, tag="inv")
                                nc.vector.tensor_tensor(
                                    sc2[:], inv[:], lam_sb[:], op=OP.mult)
                                yf = qn_pool.tile([P, 256], bf16, tag="yf")
                                nc.vector.scalar_tensor_tensor(
                                    yf[:], ys[jj][:, 0:256], sc2[:], y0s[jj][:],
                                    op0=OP.mult, op1=OP.add)
                                for e in range(2):
                                    tr_y(yt_sb[:, e, j * P:(j + 1) * P],
                                         yf[:, e * P:(e + 1) * P])

                    # ---- output projection, lagged one chunk ----
                    # Fills PE's exp-wait gaps with oproj matmuls; the lag
                    # keeps oproj off the tail of its own yt transpose chain.
                    out_eng = nc.scalar if o["out_eng"] == "scalar" else nc.sync
                    lag = o["oproj_lag"]
                    if cqi < lag:
                        obs = []
                    elif cqi == n_qchunks - 1:
                        obs = list(range(jpc * (cqi - lag), jpc * (cqi + 1)))
                    else:
                        obs = list(range(jpc * (cqi - lag), jpc * (cqi - lag + 1)))
                    for tb in obs:
                        row0 = b * t_dim + tb * P
                        merged = o["osb_merge"]
                        orow = (osb_pool.tile([P, c_dim], f16, tag="orow",
                                              name="orow")
                                if merged else None)
                        for cc in range(c_dim // 512):
                            op_ps = psum_proj.tile([P, 512], f32, tag="pp",
                                                   name="ops")
                            for e in range(2):
                                nc.tensor.matmul(
                                    op_ps[:], yt_sb[:, e, tb * P:(tb + 1) * P],
                                    wot_sb[:, e, cc * 512:(cc + 1) * 512],
                                    start=(e == 0), stop=(e == 1))
                            osb = (orow[:, cc * 512:(cc + 1) * 512] if merged
                                   else osb_pool.tile([P, 512], f16, tag="osb"))
                            oc = o["oproj_copy"]
                            if (o["tail_alt"] and b == b_dim - 1
                                    and cqi == n_qchunks - 1):
                                oc = "alt"
                            if oc == "alt":
                                oc = "act" if (tb + cc) % 2 == 0 else "dve"

                            elif oc == "rotdp":
                                oc = ("dve", "act")[(tb * 4 + cc) % 2]
                            if oc == "act":
                                nc.scalar.copy(osb, op_ps[:])
                            else:
                                nc.vector.tensor_copy(osb, op_ps[:])
                            if not merged:
                                out_eng.dma_start(
                                    out.ap()[row0:row0 + P,
                                             cc * 512:(cc + 1) * 512], osb)
                        if merged:
                            out_eng.dma_start(
                                out.ap()[row0:row0 + P, :], orow[:])

                # ---- schedule: interleave attention chunks between proj
                # chunks (deps allow A_c once proj chunk (2c+1)//(2*jpc) is
                # done; hold one extra proj chunk of slack) ----
                n_pchunks = t_dim // TOK_CHUNK
                if o["sched"] == "interleave":
                    tokens = []
                    nexta = 0
                    for ch in range(n_pchunks):
                        tokens.append(("P", ch))
                        while (nexta < n_qchunks
                               and (jpc * nexta + jpc - 1) // blocks_per_chunk
                               <= ch - 1):
                            tokens.append(("A", nexta))
                            nexta += 1
                    tokens += [("A", c) for c in range(nexta, n_qchunks)]
                else:
                    tokens = ([("P", ch) for ch in range(n_pchunks)]
                              + [("A", c) for c in range(n_qchunks)])
                for kind, idx in tokens:
                    (do_proj_chunk if kind == "P" else do_att_chunk)(idx)
    nc.compile()
    return nc


_NC_CACHE = {}
TRACE = False        # set True (e.g. from test.py) to capture an NTFF profile
LAST_RESULT = None   # BassKernelResults of the most recent run


def _get_nc(c_dim, t_dim, b_dim):
    key = (c_dim, t_dim, b_dim)
    if key not in _NC_CACHE:
        _NC_CACHE[key] = build_nc(c_dim, t_dim, b_dim)
    return _NC_CACHE[key]


def prep_inputs(x, wq, wk, wv, wo, lq1, lk1, lq2, lk2):
    """Host-side prep: per-core input maps."""
    import ml_dtypes

    bf16 = ml_dtypes.bfloat16
    fp8 = ml_dtypes.float8_e4m3
    b_dim, t_dim, c_dim = x.shape

    lam1 = np.exp(np.sum(lq1.astype(np.float64) * lk1.astype(np.float64)))
    lam2 = np.exp(np.sum(lq2.astype(np.float64) * lk2.astype(np.float64)))
    lam_full = np.float32(lam1 - lam2 + LAMBDA_INIT)

    n_ctiles = c_dim // P
    xt = np.ascontiguousarray(x.reshape(b_dim * t_dim, c_dim).T)
    xh = xt.astype(fp8)
    xl = (xt - xh.astype(np.float32)).astype(fp8)
    xt2 = np.ascontiguousarray(np.stack([xh, xl]))  # planes [hi, lo]
    lamneg = np.full((P, 1), -lam_full, dtype=np.float32)

    def part_major(a):
        """[..., 2?, C, n] -> [128, prod(rest)] with c = i*128 + p."""
        if a.ndim == 2:
            a = a[None]
        v, cdim, n = a.shape
        a = a.reshape(v, n_ctiles, P, n).transpose(2, 0, 1, 3)
        return np.ascontiguousarray(a.reshape(P, v * n_ctiles * n))

    xm = {f"xm{t}": part_major(xt2[:, :, t * P:(t + 1) * P])
          for t in range(1, TOK_CHUNK // P)}

    in_maps = []
    for h in range(N_CORES):
        sl = slice(h * HEAD_DIM, (h + 1) * HEAD_DIM)
        wk64 = wk[sl].T * np.float32(WSCALE)
        wkh = wk64.astype(fp8)
        wkl = (wk64 - wkh.astype(np.float32)).astype(fp8)
        # rampk: [x tok 0:128 | k cols] per (plane, ctile)
        rampk = part_major(np.concatenate(
            [xt2[:, :, 0:P].astype(fp8), np.stack([wkh, wkl])], axis=2))
        wqv64 = np.concatenate([wq[sl].T, wv[sl].T], axis=1) * np.float32(WSCALE)
        wqvh_ = wqv64.astype(fp8)
        wqvl_ = (wqv64 - wqvh_.astype(np.float32)).astype(fp8)
        wot_h = np.ascontiguousarray(
            (wo[:, sl] * ((1.0 - LAMBDA_INIT) / WSCALE)).T).astype(bf16)
        in_maps.append({
            "xt2": xt2, "rampk": rampk,
            "wqvh": part_major(wqvh_), "wqvl": part_major(wqvl_),
            "wot": wot_h, "lamneg": lamneg, **xm,
        })
    return in_maps


_FN_CACHE = {}


def _get_callable(nc):
    """Build (once) a reusable jitted shard_map callable for this module —
    mirrors bass2jax.run_bass_via_pjrt's multi-core path, but cached so
    repeat kernel() calls skip retracing."""
    if id(nc) in _FN_CACHE:
        return _FN_CACHE[id(nc)]
    import jax
    from jax.sharding import Mesh, PartitionSpec, NamedSharding
    from jax.experimental.shard_map import shard_map
    import concourse.mybir as mybir
    import concourse.bass2jax as b2j

    b2j.install_neuronx_cc_hook()
    pname = nc.partition_id_tensor.name if nc.partition_id_tensor else None
    in_names, out_names, out_avals, zero_shapes = [], [], [], []
    for alloc in nc.m.functions[0].allocations:
        if not isinstance(alloc, mybir.MemoryLocationSet):
            continue
        name = alloc.memorylocations[0].name
        if alloc.kind == "ExternalInput":
            if name != pname:
                in_names.append(name)
        elif alloc.kind == "ExternalOutput":
            out_names.append(name)
            shape = tuple(alloc.tensor_shape)
            dtype = mybir.dt.np(alloc.dtype)
            out_avals.append(jax.core.ShapedArray(shape, dtype))
            zero_shapes.append((shape, dtype))
    n_params = len(in_names)
    all_in = in_names + out_names
    if pname is not None:
        all_in = all_in + [pname]

    def _body(*args):
        operands = list(args)
        if pname is not None:
            operands.append(b2j.partition_id_tensor())
        return tuple(b2j._bass_exec_p.bind(
            *operands,
            out_avals=tuple(out_avals),
            in_names=tuple(all_in),
            out_names=tuple(out_names),
            lowering_input_output_aliases=(),
            sim_require_finite=True,
            sim_require_nnan=True,
            nc=nc,
        ))

    devices = jax.devices()[:N_CORES]
    mesh = Mesh(np.asarray(devices), ("core",))
    nio = n_params + len(out_names)
    fn = jax.jit(shard_map(_body, mesh=mesh,
                           in_specs=(PartitionSpec("core"),) * nio,
                           out_specs=(PartitionSpec("core"),) * len(out_names),
                           check_rep=False),
                 donate_argnums=tuple(range(n_params, nio)), keep_unused=True)
    sh = NamedSharding(mesh, PartitionSpec("core"))
    entry = (fn, in_names, out_names, zero_shapes, sh)
    _FN_CACHE[id(nc)] = entry
    return entry


def kernel(x, wq, wk, wv, wo, lq1, lk1, lq2, lk2):
    b_dim, t_dim, c_dim = x.shape
    in_maps = prep_inputs(x, wq, wk, wv, wo, lq1, lk1, lq2, lk2)
    nc = _get_nc(c_dim, t_dim, b_dim)

    try:
        import jax
        fn, in_names, out_names, zero_shapes, sh = _get_callable(nc)
        concat_in = [
            np.concatenate([np.asarray(in_maps[c][n]) for c in range(N_CORES)],
                           axis=0) for n in in_names]
        concat_zeros = [np.zeros((N_CORES * s[0], *s[1:]), d)
                        for s, d in zero_shapes]
        dev_in = [jax.device_put(a, sh) for a in concat_in]
        dev_zero = [jax.device_put(a, sh) for a in concat_zeros]
        outs = fn(*dev_in, *dev_zero)
        arr = np.asarray(outs[out_names.index("out")])
        acc = arr.reshape(N_CORES, b_dim * t_dim, c_dim).astype(
            np.float32).sum(axis=0)
    except Exception:
        from concourse.bass_utils import run_bass_kernel_spmd
        res = run_bass_kernel_spmd(nc, in_maps, list(range(N_CORES)),
                                   trace=TRACE)
        global LAST_RESULT
        LAST_RESULT = res
        acc = np.zeros((b_dim * t_dim, c_dim), dtype=np.float32)
        for h in range(N_CORES):
            acc += res.results[h]["out"].astype(np.float32)
    return acc.reshape(b_dim, t_dim, c_dim)


# revision 45
# speedup vs baseline: 1.2508x; 1.0431x over previous
"""Differential attention (B=2, T=2048, C=2048, 8 heads x 256) on 8 trn2 cores.

Sharding: tensor-parallel over the 8 effective heads — core h computes head h's
projections + attention and a partial output projection; host sums partials.

Projections run in fp8e4m3 with a 3-term residual (hi/lo planes of both x and
w, DoubleRow matmuls): (xh+xl)@(wh+wl) ~ xh@wh + (xh@wl + xl@wh), each pair of
128-contraction tiles fused into one DoubleRow instruction at 0.5 cycles/row —
25% fewer PE cycles than bf16 at bf16-level accuracy. Weights are host-scaled
by 64 into fp8's normal range; q/k absorb the scale in rms-norm (the Newton
rsqrt seed is refit for the 4096x mean-square), v stays 64x and the out-proj
weights carry 1/64.

Attention (scores S.T = K.T@Q -> exp -> P.T @ V with a ones-column giving the
softmax denominator) stays bf16: P = exp(s) spans e^-inf..e^11 which fp8
cannot represent, and bf16 q/k are needed for exp accuracy. Causal blocks
skipped; diagonal blocks masked multiplicatively post-exp.

Output is stored fp16 (halves store DMA); host sums the 8 partials in f32.
"""

import math
from contextlib import ExitStack

import numpy as np

# ---- problem constants (hardcoded per the harness contract) ----
B = 2
T = 2048
C = 2048
N_HEAD = 8
HEAD_DIM = 256
HALF = 128
LAMBDA_INIT = 0.8
RMS_EPS = 1.1920929e-07
N_CORES = 8

P = 128          # partitions
TOK_CHUNK = 512  # projection tok chunk (DMA granularity)
WSCALE = 64.0    # host-side weight scale into fp8 normal range
MSCALE = WSCALE * WSCALE  # mean-square scale (4096)

# Newton rsqrt seed: least-squares quadratic fit of rsqrt on m in
# MSCALE*[0.3, 2.0] (q/k mean-squares after the 64x weight scale), clamped.
RSQ_A = 2.07556761 / WSCALE
RSQ_B = -1.47991565 / (MSCALE * WSCALE)
RSQ_C = 0.41306651 / (MSCALE * MSCALE * WSCALE)
RSQ_CLAMP = 0.05 / WSCALE

DEFAULT_OPTS = dict(
    att_chunk=256,       # attention q-chunk width (256 or 512)
    oproj_copy="dve",    # out-proj PSUM->SBUF evac: act|dve|alt (PSUM: no pool)
    trimask_eng="dve",   # "dve" | "pool": diagonal-block mask multiply
    psum=(3, 3, 2),      # banks: (proj, st, y) — must sum to <= 8
    pt_bufs=20,          # P.T tile double-buffer depth
    xc_bufs=2,           # x chunk prefetch depth
    vcopy="act",         # "act" | "dve": V PSUM->SBUF copy engine (PSUM: no pool)
    osb_merge=True,      # one output-store DMA per tok block (vs per c-chunk)
    narrow_top=True,     # compute only the valid half of the top causal row
    tr_pool="st",        # "st" | "pp": PSUM pool used by PE transposes
    rms_bufs=4,
    qn_bufs=6,
    y0_mult=2,
    ksq_eng="dve",       # "act" | "dve": engine computing k^2
    ktcopy_eng="dve",    # "act" | "dve": engine evacuating KT psum
    ytr_pool="y",        # "st" | "y": PSUM pool for the y transposes
    ksq_src="sbuf",      # "psum" | "sbuf": k^2 input
    chunk_order="asc",   # "asc" | "desc": attention q-chunk processing order
    out_eng="scalar",    # "scalar" | "sync": queue for output stores
    osb_bufs=5,
    sched="serial",
    qk_tr="dma",
    y_tr="pe",
    tr_dma="sync",
    tail_alt=False,
    oproj_lag=3,
    oproj_rate=1,
)


def build_nc(c_dim, t_dim, b_dim, **opts):
    """Build the per-core Bass module. All shapes in tokens/channels."""
    import concourse.mybir as mybir
    import concourse.tile as tile
    from concourse import bacc
    from concourse.masks import make_identity, make_upper_triangular

    o = dict(DEFAULT_OPTS)
    o.update(opts)
    QCH = o["att_chunk"]
    jpc = QCH // P  # j-blocks per attention chunk

    dt = mybir.dt
    f32 = dt.float32
    f16 = dt.float16
    bf16 = dt.bfloat16
    fp8 = dt.float8e4
    AF = mybir.ActivationFunctionType
    OP = mybir.AluOpType
    DR = mybir.MatmulPerfMode.DoubleRow

    n_ctiles = c_dim // P            # contraction tiles over C
    npairs = n_ctiles // 2
    ntok = b_dim * t_dim             # total token rows
    n_blocks_b = t_dim // P          # 128-tok blocks per batch
    n_qchunks = t_dim // QCH         # attention q chunks per batch
    blocks_per_chunk = TOK_CHUNK // P
    inv_sqrt_half = 1.0 / math.sqrt(HALF)
    VP = 272                         # V tile pitch (256 vals + 1 ones + pad)

    nc = bacc.Bacc()
    # x and w planes are both [hi, lo]; cross-term DoubleRows pair over the
    # ctile dim so any single plane is usable as soon as its DMA lands.
    xt2 = nc.declare_dram_parameter("xt2", [2, c_dim, ntok], fp8, isOutput=False)
    # ramp tensors: host-prepacked partition-major (fully contiguous per
    # partition row) so the DMA model sees >=512B descriptors at full speed.
    # xm0..3: chunk-0 x in 128-tok pieces; wk2p: k cols both planes; wqv
    # plane-split.
    # rampk packs [x tok 0:128 | k cols] per (plane, ctile) so one DMA feeds
    # the first K-projection matmuls.
    rampk = nc.declare_dram_parameter("rampk", [P, 2 * n_ctiles * 384], fp8,
                                      isOutput=False)
    xms = {t: nc.declare_dram_parameter(f"xm{t}", [P, 2 * n_ctiles * P], fp8,
                                        isOutput=False)
           for t in range(1, blocks_per_chunk)}
    wqvh = nc.declare_dram_parameter("wqvh", [P, n_ctiles * 512], fp8,
                                     isOutput=False)
    wqvl = nc.declare_dram_parameter("wqvl", [P, n_ctiles * 512], fp8,
                                     isOutput=False)
    wot = nc.declare_dram_parameter("wot", [HEAD_DIM, c_dim], bf16, isOutput=False)
    lamneg = nc.declare_dram_parameter("lamneg", [P, 1], f32, isOutput=False)
    out = nc.declare_dram_parameter("out", [ntok, c_dim], f16, isOutput=True)

    xt_r = xt2.ap().rearrange("v (i p) t -> p v i t", p=P)    # [128,2,nct,ntok]
    wot_r = wot.ap().rearrange("(e p) n -> p e n", p=P)       # [128, 2, c_dim]

    with tile.TileContext(nc) as tc:
        with ExitStack() as ctx:
            # ---- persistent SBUF ----
            const_pool = ctx.enter_context(tc.tile_pool(name="const", bufs=1))
            # rampk tiles dim2: [x tok 0:128 | k cols 128:384]; one tile
            # per plane so a plane's consumers unblock on its own DMA.
            rampk_t = [const_pool.tile([P, n_ctiles, 384], fp8,
                                       name=f"rampk{pl}") for pl in range(2)]
            wqv_t = [const_pool.tile([P, n_ctiles, 512], fp8,
                                     name=f"wqv{pl}") for pl in range(2)]
            wot_sb = const_pool.tile([P, 2, c_dim], bf16, name="wot_sb")
            lam_sb = const_pool.tile([P, 1], f32, name="lam_sb")
            ident = const_pool.tile([P, P], bf16, name="ident")
            trimask = const_pool.tile([P, P], bf16, name="trimask")
            ones_sb = const_pool.tile([P, 1], bf16, name="ones_sb")
            nc.vector.memset(ones_sb[:], 1.0)

            xc_pool = ctx.enter_context(tc.tile_pool(name="xc", bufs=o["xc_bufs"]))

            # ---- ramp: chunk 0 comes in as part-major 128-tok minis
            # interleaved with the weight loads, so PE starts ~7us in and is
            # fed continuously. All transfers have >=512B descriptors.
            xm_t = {t: [const_pool.tile([P, n_ctiles, P], fp8,
                                        name=f"xm{t}p{pl}") for pl in range(2)]
                    for t in range(1, blocks_per_chunk)}
            rampk_r = rampk.ap().rearrange("p (v i n) -> p v i n",
                                           v=2, i=n_ctiles)
            nc.sync.dma_start(rampk_t[0][:], rampk_r[:, 0])
            nc.sync.dma_start(rampk_t[1][:], rampk_r[:, 1])
            nc.sync.dma_start(wqv_t[0][:],
                              wqvh.ap().rearrange("p (i n) -> p i n",
                                                  i=n_ctiles))
            nc.sync.dma_start(wqv_t[1][:],
                              wqvl.ap().rearrange("p (i n) -> p i n",
                                                  i=n_ctiles))
            for t in range(1, blocks_per_chunk):
                xm_r = xms[t].ap().rearrange("p (v i n) -> p v i n",
                                             v=2, i=n_ctiles)
                nc.sync.dma_start(xm_t[t][0][:], xm_r[:, 0])
                nc.sync.dma_start(xm_t[t][1][:], xm_r[:, 1])
            nc.scalar.dma_start(wot_sb[:], wot_r[:])
            nc.scalar.dma_start(lam_sb[:], lamneg.ap())
            make_identity(nc, ident[:])
            # 1.0 where kk <= q (partition <= free), else 0
            make_upper_triangular(nc, trimask[:], val=1.0, diag=True)

            qt_pool = ctx.enter_context(tc.tile_pool(name="qt", bufs=2))
            kt_pool = ctx.enter_context(tc.tile_pool(name="kt", bufs=2))
            ksq_pool = ctx.enter_context(tc.tile_pool(name="ksq", bufs=2))
            kscale_pool = ctx.enter_context(tc.tile_pool(name="kscale", bufs=2))
            v_pool = ctx.enter_context(tc.tile_pool(name="v", bufs=2))
            yt_pool = ctx.enter_context(tc.tile_pool(name="yt", bufs=2))
            pt_pool = ctx.enter_context(tc.tile_pool(name="pt", bufs=o["pt_bufs"]))
            y0_pool = ctx.enter_context(tc.tile_pool(name="y0", bufs=o["y0_mult"] * jpc))
            osb_pool = ctx.enter_context(tc.tile_pool(name="osb", bufs=o.get("osb_bufs", 3)))
            qn_pool = ctx.enter_context(tc.tile_pool(name="qn", bufs=o["qn_bufs"]))
            sq_pool = ctx.enter_context(tc.tile_pool(name="sq", bufs=2))
            rms_pool = ctx.enter_context(tc.tile_pool(name="rms", bufs=o["rms_bufs"]))
            nproj, nst, ny = o["psum"]
            psum_proj = ctx.enter_context(
                tc.tile_pool(name="psum_proj", bufs=nproj, space="PSUM"))
            psum_st = ctx.enter_context(
                tc.tile_pool(name="psum_st", bufs=nst, space="PSUM"))
            psum_y = ctx.enter_context(
                tc.tile_pool(name="psum_y", bufs=ny, space="PSUM"))

            tr_psum = psum_st if o["tr_pool"] == "st" else psum_proj
            tr_tag = o["tr_pool"] if o["tr_pool"] == "st" else "pp"
            tr_shape = QCH if o["tr_pool"] == "st" else 512

            def pe_transpose(dst_ap, src_ap):
                trp = tr_psum.tile([P, tr_shape], bf16, tag=tr_tag,
                                   name="trp")[:, :P]
                nc.tensor.transpose(trp, src_ap, ident[:])
                nc.vector.tensor_copy(dst_ap, trp)

            def pe_transpose_y(dst_ap, src_ap):
                trp = psum_y.tile([P, 257], bf16, tag="y", name="trpy")[:, :P]
                nc.tensor.transpose(trp, src_ap, ident[:])
                nc.vector.tensor_copy(dst_ap, trp)

            def dma_transpose(dst_ap, src_ap):
                tr_dma_eng = nc.sync if o["tr_dma"] == "sync" else nc.scalar
                tr_dma_eng.dma_start_transpose(out=dst_ap, in_=src_ap)

            tr_qk = pe_transpose if o["qk_tr"] == "pe" else dma_transpose
            tr_y = ((pe_transpose_y if o["ytr_pool"] == "y" else pe_transpose)
                    if o["y_tr"] == "pe" else dma_transpose)

            def dr_proj(out_ap, x_ap, w_ap, x_stationary):
                """3-term residual fp8 accumulation into out_ap [128, N].

                x_ap(pl, i): plane/ctile accessor; same for w_ap. Both
                tensors are plane [hi, lo]; cross terms pair over the ctile
                dim so each call needs only one plane of each operand. Emits
                3*npairs DoubleRow matmuls; the caller's region gets start on
                the first, stop on the last.
                """
                calls = []
                for pr in range(npairs):  # hi @ hi, k-tile pairs
                    pair = slice(2 * pr, 2 * pr + 2)
                    calls.append((x_ap(0, pair), w_ap(0, pair)))
                for pr in range(npairs):  # x_lo @ w_hi pairs
                    pair = slice(2 * pr, 2 * pr + 2)
                    calls.append((x_ap(1, pair), w_ap(0, pair)))
                for pr in range(npairs):  # x_hi @ w_lo pairs
                    pair = slice(2 * pr, 2 * pr + 2)
                    calls.append((x_ap(0, pair), w_ap(1, pair)))
                n = len(calls)
                for idx, (xs, ws) in enumerate(calls):
                    lhsT, rhs = (xs, ws) if x_stationary else (ws, xs)
                    nc.tensor.matmul(out_ap, lhsT, rhs,
                                     start=(idx == 0), stop=(idx == n - 1),
                                     perf_mode=DR)

            def rsqrt_newton(dst, m, tmp_pool, width):
                """DVE-only rsqrt on the MSCALE-shifted mean-square range.
                Returns the tile holding the result (may be a fresh tile)."""
                t1 = tmp_pool.tile([P, width], f32, tag="rms", name="rsq_t1")
                nc.vector.tensor_tensor(t1[:], m, m, op=OP.mult)
                nc.vector.tensor_scalar(dst, m, RSQ_B, RSQ_A, OP.mult, OP.add)
                nc.vector.scalar_tensor_tensor(dst, t1[:], RSQ_C, dst,
                                               op0=OP.mult, op1=OP.add)
                nc.vector.tensor_scalar_max(dst, dst, RSQ_CLAMP)
                for _ in range(2):
                    nc.vector.tensor_tensor(t1[:], dst, dst, op=OP.mult)
                    nc.vector.scalar_tensor_tensor(t1[:], t1[:], -0.5, m,
                                                   op0=OP.mult, op1=OP.mult)
                    nc.vector.tensor_scalar(t1[:], t1[:], 1.0, 1.5,
                                            OP.mult, OP.add)
                    nc.vector.tensor_tensor(dst, dst, t1[:], op=OP.mult)

            # oproj work queue: 512-col pieces, enqueued `oproj_lag`
            # attention chunks after their yt is written (across batch
            # boundaries), drained one piece per attention i-iteration.
            osteps_q = []
            yt_tiles = {}
            gchunks = b_dim * n_qchunks

            for b in range(b_dim):
                qt_sb = qt_pool.tile([P, 2, t_dim], bf16, name=f"qt_b{b}", tag="qt")
                kt_sb = kt_pool.tile([P, 2, t_dim], bf16, name=f"kt_b{b}", tag="kt")
                v_sb = v_pool.tile([P, n_blocks_b, VP], bf16, name=f"v_b{b}", tag="v")
                yt_sb = yt_pool.tile([P, 2, t_dim], bf16, name=f"yt_b{b}", tag="yt")
                yt_tiles[b] = yt_sb
                kscale_sb = kscale_pool.tile([P, 2 * n_blocks_b], f32,
                                             name=f"ksc_b{b}", tag="ksc")

                # ================= projections =================
                def do_proj_chunk(ch):
                    tok0 = b * t_dim + ch * TOK_CHUNK
                    first_ch = (b == 0 and ch == 0)
                    if first_ch:
                        xc = None
                    else:
                        xc = xc_pool.tile([P, 2, n_ctiles, TOK_CHUNK], fp8,
                                          tag="xc")
                        nc.sync.dma_start(xc[:], xt_r[:, :, :, tok0:tok0 + TOK_CHUNK])

                    # --- K projection straight into [d, tok] layout ---
                    # chunk 0 in 128-tok pieces (one per ramp mini)
                    if first_ch:
                        kpieces = [(0, P, lambda pl, i: rampk_t[pl][:, i, 0:P])]
                        for t in range(1, blocks_per_chunk):
                            kpieces.append(
                                (t * P, P,
                                 lambda pl, i, t=t: xm_t[t][pl][:, i, :]))
                    else:
                        kpieces = [
                            (h2 * 256, 256,
                             lambda pl, i, h2=h2:
                                 xc[:, pl, i, h2 * 256:(h2 + 1) * 256])
                            for h2 in range(TOK_CHUNK // 256)]
                    kssq = psum_st.tile([P, QCH], f32, tag="st",
                                        name="kssq")[:, :8]
                    ktps = [psum_proj.tile([P, 512], f32, tag="pp", name="ktp")
                            for _ in range(2)]

                    def k_piece(v, piece):
                        off, wdt, x_ap = piece
                        dr_proj(
                            ktps[v][:, off:off + wdt], x_ap,
                            lambda pl, i, v=v: rampk_t[pl][
                                :, i, P + v * P:P + (v + 1) * P],
                            x_stationary=False)

                    def k_evac():
                        for v in range(2):
                            ktp = ktps[v]
                            ktdst = kt_sb[:, v,
                                          ch * TOK_CHUNK:(ch + 1) * TOK_CHUNK]
                            if o["ktcopy_eng"] == "act":
                                nc.scalar.copy(ktdst, ktp[:])
                            else:
                                nc.vector.tensor_copy(ktdst, ktp[:])
                            ksq = ksq_pool.tile([P, TOK_CHUNK], bf16, tag="ksq")
                            ksrc = ktdst if o["ksq_src"] == "sbuf" else ktp[:]
                            if o["ksq_eng"] == "act":
                                nc.scalar.activation(ksq[:], ksrc, AF.Square)
                            elif o["ksq_eng"] == "pool":
                                nc.gpsimd.tensor_tensor(ksq[:], ksrc, ksrc,
                                                        op=OP.mult)
                            else:
                                nc.vector.tensor_tensor(ksq[:], ksrc, ksrc,
                                                        op=OP.mult)
                            for t in range(blocks_per_chunk):
                                nc.tensor.matmul(
                                    kssq[:, 2 * t + v:2 * t + v + 1],
                                    ksq[:, t * P:(t + 1) * P], ones_sb[:],
                                    start=True, stop=True)
                        # kscale = (1/64)/sqrt(mean(k^2)+eps) * inv_sqrt_half
                        ksl = kscale_sb[:, ch * 2 * blocks_per_chunk:
                                        (ch + 1) * 2 * blocks_per_chunk]
                        km = rms_pool.tile([P, 8], f32, tag="rms", name="km")
                        nc.vector.tensor_scalar(km[:], kssq[:], 1.0 / HALF,
                                                MSCALE * RMS_EPS,
                                                OP.mult, OP.add)
                        rsqrt_newton(ksl, km[:], rms_pool, 8)
                        nc.vector.tensor_scalar_mul(ksl, ksl, inv_sqrt_half)

                    if not first_ch:
                        for v in range(2):
                            for piece in kpieces:
                                k_piece(v, piece)
                        k_evac()

                    nc.vector.memset(
                        v_sb[:, ch * blocks_per_chunk:
                             (ch + 1) * blocks_per_chunk, 256:257], 1.0)
                    for tl in range(blocks_per_chunk):
                        tb = ch * blocks_per_chunk + tl
                        if first_ch:
                            # emit K piece tl just-in-time with its ramp mini
                            for v in range(2):
                                k_piece(v, kpieces[tl])
                        # one bank: [q1 q2 | v]
                        qv = psum_proj.tile([P, 512], f32, tag="pp", name="qv")
                        if first_ch:
                            xst = rampk_t if tl == 0 else xm_t[tl]
                            x_ap_qv = (lambda pl, i, xst=xst:
                                       xst[pl][:, i, 0:P])
                        else:
                            tsl = slice(tl * P, (tl + 1) * P)
                            x_ap_qv = (lambda pl, i, tsl=tsl:
                                       xc[:, pl, i, tsl])
                        for h2 in range(2):
                            dr_proj(
                                qv[:, h2 * 256:(h2 + 1) * 256], x_ap_qv,
                                lambda pl, i, h2=h2: wqv_t[pl][
                                    :, i, h2 * 256:(h2 + 1) * 256],
                                x_stationary=True)
                        halves = [qv[:, 0:128], qv[:, 128:256]]
                        rmsg = rms_pool.tile([P, 2], f32, tag="rms")
                        for j, h in enumerate(halves):
                            sq = sq_pool.tile([P, P], bf16, tag="sq", name="sq")
                            nc.scalar.activation(
                                sq[:], h, AF.Square,
                                accum_out=rmsg[:, j:j + 1])
                        nc.vector.tensor_scalar(rmsg[:], rmsg[:], 1.0 / HALF,
                                                MSCALE * RMS_EPS, OP.mult, OP.add)
                        yv = rms_pool.tile([P, 2], f32, tag="rms")
                        rsqrt_newton(yv[:], rmsg[:], rms_pool, 2)
                        for j, h in enumerate(halves):
                            qn = qn_pool.tile([P, P], bf16, tag="qn")
                            nc.vector.tensor_scalar_mul(qn[:], h, yv[:, j:j + 1])
                            tr_qk(qt_sb[:, j, tb * P:(tb + 1) * P], qn[:])
                        # V (+ ones column for the softmax denominator)
                        vsrc = qv[:, 256:512]
                        if o["vcopy"] == "act":
                            nc.scalar.copy(v_sb[:, tb, 0:256], vsrc)
                        else:
                            nc.vector.tensor_copy(v_sb[:, tb, 0:256], vsrc)
                    if first_ch:
                        k_evac()

                # ================= attention =================
                def do_att_chunk(cqi):
                    q0 = cqi * QCH
                    jmax = jpc * cqi + (jpc - 1)   # top kk-tile in this chunk

                    # ---- lagged output projection: enqueue chunk
                    # (gc - lag)'s blocks; drain one 512-col piece per
                    # i-iteration (oproj has no exp dependency, so it fills
                    # PE's exp-wait gaps).
                    out_eng = nc.scalar if o["out_eng"] == "scalar" else nc.sync
                    lag = o["oproj_lag"]
                    ncc = c_dim // 512
                    gc = b * n_qchunks + cqi
                    enq = [gc - lag] if gc >= lag else []
                    if gc == gchunks - 1:
                        enq += list(range(gc - lag + 1, gc + 1))

                    def make_ostep(gb, tb, cc, box):
                        def ostep():
                            row0 = gb * t_dim + tb * P
                            if cc == 0:
                                box["orow"] = osb_pool.tile(
                                    [P, c_dim], f16, tag="orow", name="orow")
                            orow = box["orow"]
                            op_ps = psum_proj.tile([P, 512], f32,
                                                   tag="pp", name="ops")
                            for e in range(2):
                                nc.tensor.matmul(
                                    op_ps[:],
                                    yt_tiles[gb][:, e, tb * P:(tb + 1) * P],
                                    wot_sb[:, e, cc * 512:(cc + 1) * 512],
                                    start=(e == 0), stop=(e == 1))
                            osb = orow[:, cc * 512:(cc + 1) * 512]
                            oc = o["oproj_copy"]
                            if oc == "alt":
                                oc = "act" if (tb + cc) % 2 == 0 else "dve"
                            elif oc == "rotdp":
                                oc = ("dve", "act")[(tb * ncc + cc) % 2]
                            if oc == "act":
                                nc.scalar.copy(osb, op_ps[:])
                            else:
                                nc.vector.tensor_copy(osb, op_ps[:])
                            if cc == ncc - 1:
                                out_eng.dma_start(
                                    out.ap()[row0:row0 + P, :], orow[:])
                        return ostep

                    for g in enq:
                        gb, glc = divmod(g, n_qchunks)
                        for jj in range(jpc):
                            tb = jpc * glc + jj
                            box = {}
                            for cc in range(ncc):
                                osteps_q.append(make_ostep(gb, tb, cc, box))

                    def drain_osteps(n=o["oproj_rate"]):
                        for _ in range(n):
                            if osteps_q:
                                osteps_q.pop(0)()

                    y0s = []
                    for v in range(2):
                        ys = [psum_y.tile([P, 257], f32, tag="y", name="ys")
                              for _ in range(jpc)]
                        for i in range(jmax + 1):
                            # jj0: first valid j-slot for this row (causal)
                            jj0 = max(0, i - jpc * cqi) if o["narrow_top"] else 0
                            w = QCH - jj0 * P
                            st = psum_st.tile([P, QCH], f32, tag="st",
                                              name="st")[:, :w]
                            nc.tensor.matmul(
                                st[:], kt_sb[:, v, i * P:(i + 1) * P],
                                qt_sb[:, v, q0 + jj0 * P:q0 + QCH],
                                start=True, stop=True)
                            pt = pt_pool.tile([P, QCH], bf16, tag="pt", name="pt")[:, :w]
                            nc.scalar.activation(
                                pt[:], st[:], AF.Exp,
                                scale=kscale_sb[:, 2 * i + v:2 * i + v + 1])
                            dj = i - jpc * cqi    # diagonal j-slot if >= 0
                            if dj >= 0:
                                tri_eng = (nc.gpsimd if o["trimask_eng"] == "pool"
                                           else nc.vector)
                                tri_eng.tensor_tensor(
                                    pt[:, (dj - jj0) * P:(dj - jj0 + 1) * P],
                                    pt[:, (dj - jj0) * P:(dj - jj0 + 1) * P],
                                    trimask[:], op=OP.mult)
                            for jj in range(jj0, jpc):
                                j = jpc * cqi + jj
                                if i > j:
                                    continue
                                nc.tensor.matmul(
                                    ys[jj][:],
                                    pt[:, (jj - jj0) * P:(jj - jj0 + 1) * P],
                                    v_sb[:, i, 0:257],
                                    start=(i == 0), stop=(i == j))
                            drain_osteps()
                        # epilogue for this view
                        for jj in range(jpc):
                            j = jpc * cqi + jj
                            inv = rms_pool.tile([P, 1], f32, tag="inv")
                            nc.vector.reciprocal(inv[:], ys[jj][:, 256:257])
                            if v == 0:
                                y0 = y0_pool.tile([P, 256], f32, tag="y0")
                                nc.vector.tensor_scalar_mul(
                                    y0[:], ys[jj][:, 0:256], inv[:])
                                y0s.append(y0)
                            else:
                                sc2 = rms_pool.tile([P, 1], f32, tag="inv")
                                nc.vector.tensor_tensor(
                                    sc2[:], inv[:], lam_sb[:], op=OP.mult)
                                yf = qn_pool.tile([P, 256], bf16, tag="yf")
                                nc.vector.scalar_tensor_tensor(
                                    yf[:], ys[jj][:, 0:256], sc2[:], y0s[jj][:],
                                    op0=OP.mult, op1=OP.add)
                                for e in range(2):
                                    tr_y(yt_sb[:, e, j * P:(j + 1) * P],
                                         yf[:, e * P:(e + 1) * P])

                    if gc == gchunks - 1:
                        while osteps_q:
                            osteps_q.pop(0)()

                # ---- schedule: interleave attention chunks between proj
                # chunks (deps allow A_c once proj chunk (2c+1)//(2*jpc) is
                # done; hold one extra proj chunk of slack) ----
                n_pchunks = t_dim // TOK_CHUNK
                if o["sched"] == "interleave":
                    tokens = []
                    nexta = 0
                    for ch in range(n_pchunks):
                        tokens.append(("P", ch))
                        while (nexta < n_qchunks
                               and (jpc * nexta + jpc - 1) // blocks_per_chunk
                               <= ch - 1):
                            tokens.append(("A", nexta))
                            nexta += 1
                    tokens += [("A", c) for c in range(nexta, n_qchunks)]
                else:
                    tokens = ([("P", ch) for ch in range(n_pchunks)]
                              + [("A", c) for c in range(n_qchunks)])
                for kind, idx in tokens:
                    (do_proj_chunk if kind == "P" else do_att_chunk)(idx)
    nc.compile()
    return nc


_NC_CACHE = {}
TRACE = False        # set True (e.g. from test.py) to capture an NTFF profile
LAST_RESULT = None   # BassKernelResults of the most recent run


def _get_nc(c_dim, t_dim, b_dim):
    key = (c_dim, t_dim, b_dim)
    if key not in _NC_CACHE:
        _NC_CACHE[key] = build_nc(c_dim, t_dim, b_dim)
    return _NC_CACHE[key]


def prep_inputs(x, wq, wk, wv, wo, lq1, lk1, lq2, lk2):
    """Host-side prep: per-core input maps."""
    import ml_dtypes

    bf16 = ml_dtypes.bfloat16
    fp8 = ml_dtypes.float8_e4m3
    b_dim, t_dim, c_dim = x.shape

    lam1 = np.exp(np.sum(lq1.astype(np.float64) * lk1.astype(np.float64)))
    lam2 = np.exp(np.sum(lq2.astype(np.float64) * lk2.astype(np.float64)))
    lam_full = np.float32(lam1 - lam2 + LAMBDA_INIT)

    n_ctiles = c_dim // P
    xt = np.ascontiguousarray(x.reshape(b_dim * t_dim, c_dim).T)
    xh = xt.astype(fp8)
    xl = (xt - xh.astype(np.float32)).astype(fp8)
    xt2 = np.ascontiguousarray(np.stack([xh, xl]))  # planes [hi, lo]
    lamneg = np.full((P, 1), -lam_full, dtype=np.float32)

    def part_major(a):
        """[..., 2?, C, n] -> [128, prod(rest)] with c = i*128 + p."""
        if a.ndim == 2:
            a = a[None]
        v, cdim, n = a.shape
        a = a.reshape(v, n_ctiles, P, n).transpose(2, 0, 1, 3)
        return np.ascontiguousarray(a.reshape(P, v * n_ctiles * n))

    xm = {f"xm{t}": part_major(xt2[:, :, t * P:(t + 1) * P])
          for t in range(1, TOK_CHUNK // P)}

    in_maps = []
    for h in range(N_CORES):
        sl = slice(h * HEAD_DIM, (h + 1) * HEAD_DIM)
        wk64 = wk[sl].T * np.float32(WSCALE)
        wkh = wk64.astype(fp8)
        wkl = (wk64 - wkh.astype(np.float32)).astype(fp8)
        # rampk: [x tok 0:128 | k cols] per (plane, ctile)
        rampk = part_major(np.concatenate(
            [xt2[:, :, 0:P].astype(fp8), np.stack([wkh, wkl])], axis=2))
        wqv64 = np.concatenate([wq[sl].T, wv[sl].T], axis=1) * np.float32(WSCALE)
        wqvh_ = wqv64.astype(fp8)
        wqvl_ = (wqv64 - wqvh_.astype(np.float32)).astype(fp8)
        wot_h = np.ascontiguousarray(
            (wo[:, sl] * ((1.0 - LAMBDA_INIT) / WSCALE)).T).astype(bf16)
        in_maps.append({
            "xt2": xt2, "rampk": rampk,
            "wqvh": part_major(wqvh_), "wqvl": part_major(wqvl_),
            "wot": wot_h, "lamneg": lamneg, **xm,
        })
    return in_maps


_FN_CACHE = {}


def _get_callable(nc):
    """Build (once) a reusable jitted shard_map callable for this module —
    mirrors bass2jax.run_bass_via_pjrt's multi-core path, but cached so
    repeat kernel() calls skip retracing."""
    if id(nc) in _FN_CACHE:
        return _FN_CACHE[id(nc)]
    import jax
    from jax.sharding import Mesh, PartitionSpec, NamedSharding
    from jax.experimental.shard_map import shard_map
    import concourse.mybir as mybir
    import concourse.bass2jax as b2j

    b2j.install_neuronx_cc_hook()
    pname = nc.partition_id_tensor.name if nc.partition_id_tensor else None
    in_names, out_names, out_avals, zero_shapes = [], [], [], []
    for alloc in nc.m.functions[0].allocations:
        if not isinstance(alloc, mybir.MemoryLocationSet):
            continue
        name = alloc.memorylocations[0].name
        if alloc.kind == "ExternalInput":
            if name != pname:
                in_names.append(name)
        elif alloc.kind == "ExternalOutput":
            out_names.append(name)
            shape = tuple(alloc.tensor_shape)
            dtype = mybir.dt.np(alloc.dtype)
            out_avals.append(jax.core.ShapedArray(shape, dtype))
            zero_shapes.append((shape, dtype))
    n_params = len(in_names)
    all_in = in_names + out_names
    if pname is not None:
        all_in = all_in + [pname]

    def _body(*args):
        operands = list(args)
        if pname is not None:
            operands.append(b2j.partition_id_tensor())
        return tuple(b2j._bass_exec_p.bind(
            *operands,
            out_avals=tuple(out_avals),
            in_names=tuple(all_in),
            out_names=tuple(out_names),
            lowering_input_output_aliases=(),
            sim_require_finite=True,
            sim_require_nnan=True,
            nc=nc,
        ))

    devices = jax.devices()[:N_CORES]
    mesh = Mesh(np.asarray(devices), ("core",))
    nio = n_params + len(out_names)
    fn = jax.jit(shard_map(_body, mesh=mesh,
                           in_specs=(PartitionSpec("core"),) * nio,
                           out_specs=(PartitionSpec("core"),) * len(out_names),
                           check_rep=False),
                 donate_argnums=tuple(range(n_params, nio)), keep_unused=True)
    sh = NamedSharding(mesh, PartitionSpec("core"))
    entry = (fn, in_names, out_names, zero_shapes, sh)
    _FN_CACHE[id(nc)] = entry
    return entry


def kernel(x, wq, wk, wv, wo, lq1, lk1, lq2, lk2):
    b_dim, t_dim, c_dim = x.shape
    in_maps = prep_inputs(x, wq, wk, wv, wo, lq1, lk1, lq2, lk2)
    nc = _get_nc(c_dim, t_dim, b_dim)

    try:
        import jax
        fn, in_names, out_names, zero_shapes, sh = _get_callable(nc)
        concat_in = [
            np.concatenate([np.asarray(in_maps[c][n]) for c in range(N_CORES)],
                           axis=0) for n in in_names]
        concat_zeros = [np.zeros((N_CORES * s[0], *s[1:]), d)
                        for s, d in zero_shapes]
        dev_in = [jax.device_put(a, sh) for a in concat_in]
        dev_zero = [jax.device_put(a, sh) for a in concat_zeros]
        outs = fn(*dev_in, *dev_zero)
        arr = np.asarray(outs[out_names.index("out")])
        acc = arr.reshape(N_CORES, b_dim * t_dim, c_dim).astype(
            np.float32).sum(axis=0)
    except Exception:
        from concourse.bass_utils import run_bass_kernel_spmd
        res = run_bass_kernel_spmd(nc, in_maps, list(range(N_CORES)),
                                   trace=TRACE)
        global LAST_RESULT
        LAST_RESULT = res
        acc = np.zeros((b_dim * t_dim, c_dim), dtype=np.float32)
        for h in range(N_CORES):
            acc += res.results[h]["out"].astype(np.float32)
    return acc.reshape(b_dim, t_dim, c_dim)
